# revision 14
# baseline (speedup 1.0000x reference)
# Trainium2 Bass kernel for nn_CrossDeformAttn (deformable cross attention).
#
# Math (per batch b):
#   xc = x^T                                  (D, L) channel-major
#   q  = Wq xc + bq                           (D, L)
#   offset path: conv(q grouped, Woff1) -> Woff2 -> tanh*K -> bilinear sample
#     positions, sample xc per group -> xs    (D, L)
#   k = Wk xs + bk ; v = Wv xs + bv + rel_bias
#   per head (64 ch):  sT = k_h^T q_h summed over L   (64x64)
#   attn = softmax_j(sT * D^-0.5) ; out_h = attn @ v_h ; y = out^T Wo^T + bo
#
# Key host-side folding: the whole offset conv collapses.  With
#   Weff[i,t] = sum_o Woff2[o] Woff1[o,i,t]        (256, 5)
#   wproj[g,t,:] = Weff[:,t]^T @ Wq[g*256:(g+1)*256, :]   -> Poff (D, 20)
# the pre-tanh offset signal is a 20-channel projection of x plus shifted
# sums -- no 85 GFLOP conv on device.
#
# Device pipeline per core (1 batch element per core, 8 cores):
#   pass 1: stream x chunks -> PE-transpose -> xc -> qT (token-major, spill to
#           DRAM) + off_pre (20, L) -> shifted-tap accumulate (DMA accum)
#   offset math: tanh -> positions -> floor/frac (via fmod) -> clamped int16
#           indices (16-wrapped via tiny PE transposes) + bilinear weights
#           (token-major via PE transposes)
#   pass 2: dma_gather rows of x from HBM (2 taps x 4 groups), bilinear
#           combine token-major, PE-transpose -> xs, K GEMM -> kT, V GEMM ->
#           v (+rel_bias, spill), score matmuls accumulate in PSUM
#   softmax: exp on ACT (no max-sub needed; logits ~ +-5), row sums via
#           matmul with ones, reciprocal
#   pass 3: out_h = E^T.T @ v_h scaled by 1/rowsum, final Wo GEMM -> y
#
# All GEMMs run with float32r (FP22 reads, full PE rate at N>=256).

import sys

if "/opt/trn_rl_repo" not in sys.path:
    sys.path.insert(0, "/opt/trn_rl_repo")

from contextlib import ExitStack

import numpy as np

import concourse.bass as bass
import concourse.tile as tile
from concourse import bacc
from concourse import mybir
from concourse.bass_utils import run_bass_kernel_spmd
from concourse.masks import make_identity

f32 = mybir.dt.float32
f32r = mybir.dt.float32r
bf16 = mybir.dt.bfloat16
i16 = mybir.dt.int16
Alu = mybir.AluOpType
Act = mybir.ActivationFunctionType

B, L, D = 8, 4096, 1024
H, G, KW = 16, 4, 5
DG, DH = D // G, D // H
NCORES = 8

C1 = 512            # pass-1 chunk (tokens)
NC1 = L // C1
C2 = 256            # pass-2 chunk
NC2 = L // C2
C3 = 512            # pass-3 chunk
NC3 = L // C3

S_NORM = float(L) / float(L + 3)   # position scale from grid_sample algebra
ATT_SCALE = 1.0 / float(D) ** 0.5


def build_program(has_bq: bool, has_bk: bool, has_bo: bool) -> bass.Bass:
    nc = bacc.Bacc("TRN2", debug=False)

    x_e = nc.declare_dram_parameter("x", [L, D], f32, isOutput=False)
    wq_e = nc.declare_dram_parameter("WqTs", [128, 8, D], f32r, isOutput=False)
    wk_e = nc.declare_dram_parameter("WkTs", [128, 8, D], f32r, isOutput=False)
    wv_e = nc.declare_dram_parameter("WvTs", [128, 8, D], f32r, isOutput=False)
    wo_e = nc.declare_dram_parameter("WoTs", [128, 8, D], f32r, isOutput=False)
    poff_e = nc.declare_dram_parameter("Poffs", [128, 8, 128], f32r, isOutput=False)
    ones_e = nc.declare_dram_parameter("ones2", [128, 2], f32r, isOutput=False)
    offb_e = nc.declare_dram_parameter("offbias", [20, 1], f32, isOutput=False)
    relb_e = nc.declare_dram_parameter("relb", [128, 8, L], bf16, isOutput=False)
    ios_e = nc.declare_dram_parameter("ios16", [4, L], f32, isOutput=False)
    c0_e = nc.declare_dram_parameter("c0vec", [128, 1], f32, isOutput=False)
    pad2_e = nc.declare_dram_parameter("pad2", [4, 2], f32, isOutput=False)
    if has_bq:
        bq_e = nc.declare_dram_parameter("bqrep", [128, D], f32, isOutput=False)
    if has_bk:
        bk_e = nc.declare_dram_parameter("bkrep", [128, D], f32, isOutput=False)
    if has_bo:
        bo_e = nc.declare_dram_parameter("borep", [128, D], f32, isOutput=False)
    y_e = nc.declare_dram_parameter("y", [L, D], f32, isOutput=True)

    qT_d = nc.dram_tensor("qT_scratch", [L, D], f32r)
    v_d = nc.dram_tensor("v_scratch", [128, 8, L], f32r)

    with tile.TileContext(nc) as tc, ExitStack() as ctx:
        singles = ctx.enter_context(tc.tile_pool(name="singles", bufs=1))
        wpool = ctx.enter_context(tc.tile_pool(name="wpool", bufs=2))
        ps_t = ctx.enter_context(tc.tile_pool(name="ps_t", bufs=2, space="PSUM"))
        ps_g = ctx.enter_context(tc.tile_pool(name="ps_g", bufs=2, space="PSUM"))
        ps_v = ctx.enter_context(tc.tile_pool(name="ps_v", bufs=2, space="PSUM"))
        ps_s = ctx.enter_context(tc.tile_pool(name="ps_s", bufs=1, space="PSUM"))

        ident = singles.tile([128, 128], f32, name="ident")
        make_identity(nc, ident)

        wq = wpool.tile([128, 8, D], f32r, tag="w", name="wq")
        nc.scalar.dma_start(out=wq, in_=wq_e[:])
        poffs = singles.tile([128, 8, 128], f32r, name="poffs")
        ones2 = singles.tile([128, 2], f32r, name="ones2")
        nc.sync.dma_start(out=ones2, in_=ones_e[:])
        nc.sync.dma_start(out=poffs, in_=poff_e[:])
        offb = singles.tile([20, 1], f32, name="offb")
        nc.sync.dma_start(out=offb, in_=offb_e[:])
        c0v = singles.tile([128, 1], f32, name="c0v")
        nc.sync.dma_start(out=c0v, in_=c0_e[:])
        if has_bq:
            bqr = singles.tile([128, D], f32, name="bqr")
            nc.sync.dma_start(out=bqr, in_=bq_e[:])
        if has_bk:
            bkr = singles.tile([128, D], f32, name="bkr")
            nc.sync.dma_start(out=bkr, in_=bk_e[:])
        if has_bo:
            bor = singles.tile([128, D], f32, name="bor")
            nc.sync.dma_start(out=bor, in_=bo_e[:])

        # pre-tanh offset accumulator over the 5 conv taps (pad 8 absorbs the
        # shifted accumulate windows; cols >= L are spillover junk)
        offraw = singles.tile([4, L + 8], f32, name="offraw")
        nc.vector.memset(offraw[:], 0.0)

        wtok = singles.tile([128, 32, 8], f32, name="wtok")
        idxw = singles.tile([128, NC2, 8, C2 // 16], i16, name="idxw")
        # separate PSUM banks for even/odd heads so each bank has exactly one
        # accumulation-group start/stop
        score_e = ps_s.tile([128, 512], f32, tag="score_e", name="score_e")
        score_o = ps_s.tile([128, 512], f32, tag="score_o", name="score_o")

        # ------------------------- PASS 1 -------------------------
        with tc.tile_pool(name="p1", bufs=2) as p1, \
             tc.tile_pool(name="p1o", bufs=2) as p1o:
            for c in range(NC1):
                l0 = c * C1
                xt = p1.tile([128, C1 // 128, D], f32, tag="xt", name="xt")
                nc.sync.dma_start(
                    out=xt[:],
                    in_=x_e[l0 : l0 + C1, :].rearrange("(ls p) d -> p ls d", p=128),
                )
                xc = p1.tile([128, 8, C1], f32r, tag="xc", name="xc")
                for dc in range(8):
                    pt = ps_t.tile([128, 512], f32, tag="t", name="pt")
                    for ls in range(C1 // 128):
                        nc.tensor.transpose(
                            pt[:, ls * 128 : (ls + 1) * 128],
                            xt[:, ls, dc * 128 : (dc + 1) * 128],
                            ident[:],
                        )
                    nc.scalar.activation(out=xc[:, dc, :], in_=pt[:], func=Act.Copy)
                qt = p1.tile([128, C1 // 128, D], f32r, tag="qt", name="qt")
                for ls in range(C1 // 128):
                    for oc in range(2):
                        pg = ps_g.tile([128, 512], f32, tag="g", name="pg")
                        for dc in range(8):
                            nc.tensor.matmul(
                                pg[:],
                                lhsT=xc[:, dc, ls * 128 : (ls + 1) * 128],
                                rhs=wq[:, dc, oc * 512 : (oc + 1) * 512],
                                start=(dc == 0),
                                stop=(dc == 7),
                            )
                        nc.vector.tensor_copy(
                            out=qt[:, ls, oc * 512 : (oc + 1) * 512], in_=pg[:]
                        )
                if has_bq:
                    for ls in range(C1 // 128):
                        nc.vector.tensor_tensor(
                            out=qt[:, ls, :], in0=qt[:, ls, :], in1=bqr[:], op=Alu.add
                        )
                nc.scalar.dma_start(
                    out=qT_d[l0 : l0 + C1, :].rearrange("(ls p) o -> p ls o", p=128),
                    in_=qt[:],
                )
                # off_pre: (20, C1) = Poff^T @ xc_chunk
                po = ps_v.tile([128, 512], f32, tag="v", name="po")
                for dc in range(8):
                    nc.tensor.matmul(
                        po[:, :C1],
                        lhsT=poffs[:, dc, :],
                        rhs=xc[:, dc, :],
                        start=(dc == 0),
                        stop=(dc == 7),
                    )
                offsb = p1o.tile([20, C1], f32, tag="offsb", name="offsb")
                nc.scalar.activation(
                    out=offsb[:], in_=po[:20, :C1], func=Act.Identity,
                    bias=offb[:], scale=1.0,
                )
                # shifted tap accumulate: off_pre[4t+g, l'] adds into
                # offraw[g, l' + 4 - t]
                for t in range(KW):
                    nc.gpsimd.dma_start(
                        out=offraw[0:4, l0 + 4 - t : l0 + 4 - t + C1],
                        in_=offsb[4 * t : 4 * t + 4, :],
                        accum_op=Alu.add,
                    )

        wk = wpool.tile([128, 8, D], f32r, tag="w", name="wk")
        nc.scalar.dma_start(out=wk, in_=wk_e[:])
        wv = wpool.tile([128, 8, D], f32r, tag="w", name="wv")
        nc.scalar.dma_start(out=wv, in_=wv_e[:])

        # ------------------------- OFFSET MATH -------------------------
        # Fold [4, L] -> [128, 128] (partition = g*32 + l//128) so the
        # elementwise chain uses all DVE lanes; fold/unfold are cheap DMAs.
        # arri rows: 0-3 = i0 clamped per group, 4-7 = i1 clamped
        # arrw rows: 0-3 = w0' (=(1-frac)*valid0), 4-7 = w1' (=frac*valid1)
        with tc.tile_pool(name="om", bufs=1) as om, \
             tc.tile_pool(name="omb", bufs=1) as omb:
            arri = omb.tile([8, L], f32, name="arri")
            arrw = omb.tile([8, L], f32, name="arrw")
            zf = om.tile([128, 128], f32, tag="zf", name="zf")
            nc.sync.dma_start(
                out=zf[:],
                in_=offraw[0:4, 0:L].rearrange("g (lb li) -> g lb li", li=128),
            )
            iosb = om.tile([128, 128], f32, tag="iosb", name="iosb")
            nc.sync.dma_start(out=iosb[:], in_=ios_e[:].rearrange(
                "g (lb li) -> g lb li", li=128))
            nc.vector.tensor_scalar(zf[:], zf[:], c0v[:, 0:1], None, Alu.add)
            # reference: offset for l<2 is exactly boff2 (the concat pad);
            # l in {0,1} sits at partitions g*32, cols 0:2
            for g in range(G):
                nc.sync.dma_start(out=zf[32 * g : 32 * g + 1, 0:2],
                                  in_=pad2_e[g : g + 1, :])
            th = om.tile([128, 128], f32, tag="th", name="th")
            nc.scalar.activation(out=th[:], in_=zf[:], func=Act.Tanh)
            posp = om.tile([128, 128], f32, tag="posp", name="posp")
            # posp = 16 + pos = KW*tanh(z)*S + (iota*S - 0.5 + 16)
            nc.vector.tensor_scalar(posp[:], th[:], float(KW) * S_NORM, None,
                                    Alu.mult)
            nc.vector.tensor_tensor(out=posp[:], in0=posp[:], in1=iosb[:],
                                    op=Alu.add)
            i16t = om.tile([128, 128], i16, tag="i16t", name="i16t")
            nc.vector.tensor_copy(out=i16t[:], in_=posp[:])
            flo = om.tile([128, 128], f32, tag="flo", name="flo")
            nc.vector.tensor_copy(out=flo[:], in_=i16t[:])
            # rounding-mode-agnostic floor: flo -= (flo > posp)
            corr = om.tile([128, 128], f32, tag="corr", name="corr")
            nc.vector.tensor_tensor(out=corr[:], in0=flo[:], in1=posp[:],
                                    op=Alu.subtract)
            nc.vector.tensor_scalar(corr[:], corr[:], float(2 ** 25), 1.0,
                                    Alu.mult, Alu.min)
            nc.vector.tensor_scalar(corr[:], corr[:], 0.0, None, Alu.max)
            nc.vector.tensor_tensor(out=flo[:], in0=flo[:], in1=corr[:],
                                    op=Alu.subtract)
            w1 = om.tile([128, 128], f32, tag="w1", name="w1")
            nc.vector.tensor_tensor(out=w1[:], in0=posp[:], in1=flo[:],
                                    op=Alu.subtract)
            i0f = om.tile([128, 128], f32, tag="i0f", name="i0f")
            nc.vector.tensor_scalar(i0f[:], flo[:], -16.0, None, Alu.add)
            # valid masks from clips (integer-valued i0f):
            v0 = om.tile([128, 128], f32, tag="v0", name="v0")
            nc.vector.tensor_scalar(v0[:], i0f[:], 1.0, 1.0, Alu.add, Alu.min)
            nc.vector.tensor_scalar(v0[:], v0[:], 0.0, None, Alu.max)
            vtmp = om.tile([128, 128], f32, tag="vtmp", name="vtmp")
            nc.vector.tensor_scalar(vtmp[:], i0f[:], -1.0, 4096.0, Alu.mult,
                                    Alu.add)
            nc.vector.tensor_scalar(vtmp[:], vtmp[:], 1.0, 0.0, Alu.min, Alu.max)
            nc.vector.tensor_tensor(out=v0[:], in0=v0[:], in1=vtmp[:],
                                    op=Alu.mult)
            u0 = om.tile([128, 128], f32, tag="u0", name="u0")
            nc.vector.tensor_scalar(u0[:], i0f[:], 2.0, 1.0, Alu.add, Alu.min)
            nc.vector.tensor_scalar(u0[:], u0[:], 0.0, None, Alu.max)
            nc.vector.tensor_scalar(vtmp[:], i0f[:], -1.0, 4095.0, Alu.mult,
                                    Alu.add)
            nc.vector.tensor_scalar(vtmp[:], vtmp[:], 1.0, 0.0, Alu.min, Alu.max)
            nc.vector.tensor_tensor(out=u0[:], in0=u0[:], in1=vtmp[:],
                                    op=Alu.mult)
            w1p = om.tile([128, 128], f32, tag="w1p", name="w1p")
            nc.vector.tensor_tensor(out=w1p[:], in0=w1[:], in1=u0[:],
                                    op=Alu.mult)
            w0p = om.tile([128, 128], f32, tag="w0p", name="w0p")
            nc.vector.tensor_scalar(w0p[:], w1[:], -1.0, 1.0, Alu.mult, Alu.add)
            nc.vector.tensor_tensor(out=w0p[:], in0=w0p[:], in1=v0[:],
                                    op=Alu.mult)
            i0c = om.tile([128, 128], f32, tag="i0c", name="i0c")
            nc.vector.tensor_scalar(i0c[:], i0f[:], 0.0, 4095.0, Alu.max,
                                    Alu.min)
            i1c = om.tile([128, 128], f32, tag="i1c", name="i1c")
            nc.vector.tensor_scalar(i1c[:], i0f[:], 1.0, 0.0, Alu.add, Alu.max)
            nc.vector.tensor_scalar(i1c[:], i1c[:], 4095.0, None, Alu.min)
            # unfold into 8-row stacks for the PE wrap/weight transposes
            for srcf, dst, r0 in ((i0c, arri, 0), (i1c, arri, 4),
                                  (w0p, arrw, 0), (w1p, arrw, 4)):
                nc.sync.dma_start(
                    out=dst[r0 : r0 + 4, :].rearrange(
                        "g (lb li) -> g lb li", li=128),
                    in_=srcf[:],
                )

            # 16-wrapped int16 index layout for dma_gather: idx set r=(tap*4+g),
            # wrapped[p, k] = value at token 16k+p
            for pt4 in range(4):
                pw = ps_t.tile([16, 512], f32, tag="t", name="pw")
                for kk in range(64):
                    k = pt4 * 64 + kk
                    nc.tensor.transpose(
                        pw[:, kk * 8 : (kk + 1) * 8],
                        arri[:, k * 16 : (k + 1) * 16],
                        ident[0:8, 0:8],
                    )
                # chunks of C2 tokens = 16 k-blocks; 4 chunks per psum tile
                nc.vector.tensor_copy(
                    out=idxw[0:16, pt4 * 4 : (pt4 + 1) * 4, :, :],
                    in_=pw[:].rearrange("p (c kk st) -> p c st kk", c=4, kk=16,
                                        st=8),
                )
            for r in range(1, 8):
                nc.sync.dma_start(out=idxw[16 * r : 16 * (r + 1)], in_=idxw[0:16])

            # token-major bilinear weights: wtok[p, B, r] = arrw[r, 128B + p]
            pww = ps_t.tile([128, 512], f32, tag="t", name="pww")
            for b in range(32):
                nc.tensor.transpose(
                    pww[:, b * 8 : (b + 1) * 8],
                    arrw[:, b * 128 : (b + 1) * 128],
                    ident[0:8, 0:8],
                )
            nc.vector.tensor_copy(out=wtok[:], in_=pww[:, 0:256])

        # ------------------------- PASS 2 -------------------------
        nidx_reg = nc.gpsimd.to_reg(C2)
        with tc.tile_pool(name="p2", bufs=2) as p2, \
             tc.tile_pool(name="p2b", bufs=1) as p2b, \
             tc.tile_pool(name="p2g", bufs=4) as p2g:
            for c in range(NC2):
                l0 = c * C2
                qt2 = p2.tile([128, C2 // 128, D], f32r, tag="qt2", name="qt2")
                nc.scalar.dma_start(
                    out=qt2[:],
                    in_=qT_d[l0 : l0 + C2, :].rearrange("(ls p) o -> p ls o", p=128),
                )
                relb = p2.tile([128, 8, C2], bf16, tag="relb", name="relb")
                nc.scalar.dma_start(out=relb[:], in_=relb_e[:, :, l0 : l0 + C2])
                xsT = p2.tile([128, C2 // 128, D], f32, tag="xsT", name="xsT")
                for g in range(G):
                    ga = p2g.tile([128, C2 // 128, 256], f32, tag="gth", name="ga")
                    nc.gpsimd.dma_gather(
                        out_ap=ga[:],
                        in_ap=x_e[:, g * 256 : (g + 1) * 256],
                        idxs_ap=idxw[:, c, g, :],
                        num_idxs=C2,
                        num_idxs_reg=nidx_reg,
                        elem_size=256,
                        elem_step=D,
                    )
                    gb = p2g.tile([128, C2 // 128, 256], f32, tag="gth", name="gb")
                    nc.gpsimd.dma_gather(
                        out_ap=gb[:],
                        in_ap=x_e[:, g * 256 : (g + 1) * 256],
                        idxs_ap=idxw[:, c, 4 + g, :],
                        num_idxs=C2,
                        num_idxs_reg=nidx_reg,
                        elem_size=256,
                        elem_step=D,
                    )
                    nb = C2 // 128
                    wa = wtok[:, nb * c : nb * (c + 1), g, None].to_broadcast(
                        (128, nb, 256))
                    wb = wtok[:, nb * c : nb * (c + 1), 4 + g, None].to_broadcast(
                        (128, nb, 256))
                    nc.vector.tensor_tensor(out=ga[:], in0=ga[:], in1=wa, op=Alu.mult)
                    nc.vector.tensor_tensor(out=gb[:], in0=gb[:], in1=wb, op=Alu.mult)
                    nc.vector.tensor_tensor(
                        out=xsT[:, :, g * 256 : (g + 1) * 256], in0=ga[:], in1=gb[:],
                        op=Alu.add,
                    )
                xs = p2b.tile([128, 8, C2], f32r, tag="xs", name="xs")
                for dc in range(8):
                    pt = ps_t.tile([128, 512], f32, tag="t", name="pt2")
                    for ls in range(C2 // 128):
                        nc.tensor.transpose(
                            pt[:, ls * 128 : (ls + 1) * 128],
                            xsT[:, ls, dc * 128 : (dc + 1) * 128],
                            ident[:],
                        )
                    nc.scalar.activation(out=xs[:, dc, :], in_=pt[:, 0:C2],
                                         func=Act.Copy)
                kt = p2b.tile([128, C2 // 128, D], f32r, tag="kt", name="kt")
                for ls in range(C2 // 128):
                    for oc in range(2):
                        pg = ps_g.tile([128, 512], f32, tag="g", name="pg2")
                        for dc in range(8):
                            nc.tensor.matmul(
                                pg[:],
                                lhsT=xs[:, dc, ls * 128 : (ls + 1) * 128],
                                rhs=wk[:, dc, oc * 512 : (oc + 1) * 512],
                                start=(dc == 0),
                                stop=(dc == 7),
                            )
                        nc.scalar.activation(
                            out=kt[:, ls, oc * 512 : (oc + 1) * 512], in_=pg[:],
                            func=Act.Copy,
                        )
                if has_bk:
                    for ls in range(C2 // 128):
                        nc.vector.tensor_tensor(
                            out=kt[:, ls, :], in0=kt[:, ls, :], in1=bkr[:], op=Alu.add
                        )
                vsb = p2.tile([128, 8, C2], f32r, tag="vsb", name="vsb")
                for dc in range(8):
                    pv = ps_v.tile([128, 512], f32, tag="v", name="pv")
                    for ds in range(8):
                        nc.tensor.matmul(
                            pv[:, :C2],
                            lhsT=wv[:, ds, dc * 128 : (dc + 1) * 128],
                            rhs=xs[:, ds, :],
                            start=(ds == 0),
                            stop=(ds == 7),
                        )
                    nc.vector.tensor_tensor(
                        out=vsb[:, dc, :], in0=pv[:, :C2], in1=relb[:, dc, :],
                        op=Alu.add,
                    )
                nc.scalar.dma_start(out=v_d[:, :, l0 : l0 + C2], in_=vsb[:])
                # scores: sT[j, i] accumulated across all chunks in one PSUM bank
                # head-pair blocks: pair p covers channels [128p, 128p+128);
                # the [128,128] block has the two per-head scores on its
                # diagonal 64-blocks (off-diagonals are junk, zeroed later)
                for ls in range(C2 // 128):
                    for p in range(8):
                        sbank = score_e if p < 4 else score_o
                        pc = (p % 4) * 128
                        nc.tensor.matmul(
                            sbank[:, pc : pc + 128],
                            lhsT=kt[:, ls, 128 * p : 128 * p + 128],
                            rhs=qt2[:, ls, 128 * p : 128 * p + 128],
                            start=(c == 0 and ls == 0 and p % 4 == 0),
                            stop=(c == NC2 - 1 and ls == C2 // 128 - 1
                                  and p % 4 == 3),
                        )

        wo = wpool.tile([128, 8, D], f32r, tag="w", name="wo")
        nc.scalar.dma_start(out=wo, in_=wo_e[:])

        # ------------------------- SOFTMAX -------------------------
        esb = singles.tile([128, 1024], f32r, name="esb")
        nc.vector.memset(esb[:].bitcast(f32), 0.0)
        for h in range(H):
            p = h // 2
            sbank = score_e if p < 4 else score_o
            pc = (p % 4) * 128
            r0 = 64 * (h % 2)
            nc.scalar.activation(
                out=esb[r0 : r0 + 64, 128 * p + r0 : 128 * p + r0 + 64],
                in_=sbank[r0 : r0 + 64, pc + r0 : pc + r0 + 64],
                func=Act.Exp, scale=ATT_SCALE,
            )
        rs_ps = ps_g.tile([128, 16], f32, tag="g", name="rs_ps")
        for p in range(8):
            nc.tensor.matmul(
                rs_ps[:, 2 * p : 2 * p + 2],
                lhsT=esb[:, 128 * p : 128 * p + 128],
                rhs=ones2[:],
                start=True,
                stop=True,
            )
        rsi = singles.tile([128, 16], f32, name="rsi")
        nc.vector.reciprocal(out=rsi[:], in_=rs_ps[:])

        # ------------------------- PASS 3 -------------------------
        with tc.tile_pool(name="p3", bufs=2) as p3, tc.tile_pool(name="p3b", bufs=1) as p3b:
            for c in range(NC3):
                l0 = c * C3
                vs3 = p3.tile([128, 8, C3], f32r, tag="v3", name="vs3")
                nc.scalar.dma_start(out=vs3[:], in_=v_d[:, :, l0 : l0 + C3])
                osb = p3b.tile([128, 8, C3], f32r, tag="o3", name="osb")
                for p in range(8):
                    po3 = ps_v.tile([128, 512], f32, tag="v", name="po3")
                    nc.tensor.matmul(
                        po3[:],
                        lhsT=esb[:, 128 * p : 128 * p + 128],
                        rhs=vs3[:, p, :],
                        start=True,
                        stop=True,
                    )
                    nc.scalar.activation(
                        out=osb[:, p, :], in_=po3[:], func=Act.Identity,
                        bias=0.0, scale=rsi[:, 2 * p : 2 * p + 1],
                    )
                yt = p3.tile([128, C3 // 128, D], f32, tag="yt", name="yt")
                for ls in range(C3 // 128):
                    for oc in range(2):
                        pg = ps_g.tile([128, 512], f32, tag="g", name="pg3")
                        for dc in range(8):
                            nc.tensor.matmul(
                                pg[:],
                                lhsT=osb[:, dc, ls * 128 : (ls + 1) * 128],
                                rhs=wo[:, dc, oc * 512 : (oc + 1) * 512],
                                start=(dc == 0),
                                stop=(dc == 7),
                            )
                        nc.vector.tensor_copy(
                            out=yt[:, ls, oc * 512 : (oc + 1) * 512], in_=pg[:]
                        )
                if has_bo:
                    for ls in range(C3 // 128):
                        nc.vector.tensor_tensor(
                            out=yt[:, ls, :], in0=yt[:, ls, :], in1=bor[:], op=Alu.add
                        )
                nc.sync.dma_start(
                    out=y_e[l0 : l0 + C3, :].rearrange("(ls p) o -> p ls o", p=128),
                    in_=yt[:],
                )

    nc.compile()
    return nc


_prog_cache: dict = {}


def get_program(has_bq: bool, has_bk: bool, has_bo: bool) -> bass.Bass:
    key = (has_bq, has_bk, has_bo)
    if key not in _prog_cache:
        _prog_cache[key] = build_program(*key)
    return _prog_cache[key]


def make_in_maps(inputs: dict) -> tuple[list[dict], tuple]:
    import ml_dtypes

    x = np.ascontiguousarray(np.asarray(inputs["x"], np.float32))
    Wq = np.asarray(inputs["Wq"], np.float32)
    bq = np.asarray(inputs["bq"], np.float32)
    Wk = np.asarray(inputs["Wk"], np.float32)
    bk = np.asarray(inputs["bk"], np.float32)
    Wv = np.asarray(inputs["Wv"], np.float32)
    bv = np.asarray(inputs["bv"], np.float32)
    Wo = np.asarray(inputs["Wo"], np.float32)
    bo = np.asarray(inputs["bo"], np.float32)
    Woff1 = np.asarray(inputs["Woff1"], np.float32)
    boff1 = np.asarray(inputs["boff1"], np.float32)
    Woff2 = np.asarray(inputs["Woff2"], np.float32)
    boff2 = np.asarray(inputs["boff2"], np.float32)
    rel_bias = np.asarray(inputs["rel_bias"], np.float32)

    def wts(w):  # (D, D) weight -> lhsT layout [128, 8, D]: [p, dc, o] = w[o, dc*128+p]
        return np.ascontiguousarray(
            w.T.reshape(8, 128, D).transpose(1, 0, 2).astype(np.float32)
        )

    # offset-path folding
    Weff = np.einsum("o,oit->it", Woff2[0].astype(np.float64),
                     Woff1.astype(np.float64))            # (256, 5)
    Poff = np.zeros((D, 128), np.float64)                 # padded to M=128 for fp32r
    offbias = np.zeros((20, 1), np.float64)
    for t in range(KW):
        for g in range(G):
            r = 4 * t + g
            Poff[:, r] = Weff[:, t] @ Wq[g * DG : (g + 1) * DG, :].astype(np.float64)
            offbias[r, 0] = Weff[:, t] @ bq[g * DG : (g + 1) * DG].astype(np.float64)
    c0 = float(Woff2[0].astype(np.float64) @ boff1.astype(np.float64)
               + np.float64(boff2[0]))
    Poffs = np.ascontiguousarray(
        Poff.reshape(8, 128, 128).transpose(1, 0, 2).astype(np.float32)
    )

    rel_eff = rel_bias[0] + bv[:, None]                   # (D, L)
    relb = np.ascontiguousarray(
        rel_eff.reshape(8, 128, L).transpose(1, 0, 2).astype(ml_dtypes.bfloat16)
    )

    ios16 = np.tile(
        (np.arange(L, dtype=np.float64) * S_NORM - 0.5 + 16.0)[None, :], (4, 1)
    ).astype(np.float32)
    c0vec = np.full((128, 1), c0, np.float32)
    pad2 = np.full((4, 2), float(boff2[0]), np.float32)

    has_bq = bool(np.any(bq != 0.0))
    has_bk = bool(np.any(bk != 0.0))
    has_bo = bool(np.any(bo != 0.0))

    shared = {
        "WqTs": wts(Wq),
        "WkTs": wts(Wk),
        "WvTs": wts(Wv),
        "WoTs": wts(Wo),
        "Poffs": Poffs,
        "offbias": offbias.astype(np.float32),
        "relb": relb,
        "ios16": ios16,
        "c0vec": c0vec,
        "pad2": pad2,
        "ones2": np.ones((128, 2), np.float32),
    }
    if has_bq:
        shared["bqrep"] = np.ascontiguousarray(np.tile(bq[None, :], (128, 1)))
    if has_bk:
        shared["bkrep"] = np.ascontiguousarray(np.tile(bk[None, :], (128, 1)))
    if has_bo:
        shared["borep"] = np.ascontiguousarray(np.tile(bo[None, :], (128, 1)))

    in_maps = [dict(shared, x=np.ascontiguousarray(x[i])) for i in range(NCORES)]
    return in_maps, (has_bq, has_bk, has_bo)


def kernel_run(inputs: dict, trace: bool = False):
    in_maps, flags = make_in_maps(inputs)
    nc = get_program(*flags)
    res = run_bass_kernel_spmd(nc, in_maps, list(range(NCORES)), trace=trace)
    y = np.stack([res.results[i]["y"] for i in range(NCORES)], axis=0)
    return y, res


def kernel(**inputs) -> np.ndarray:
    y, _ = kernel_run(inputs, trace=False)
    return y


# revision 15
# speedup vs baseline: 1.1596x; 1.1596x over previous
# Trainium2 Bass kernel for nn_CrossDeformAttn (deformable cross attention).
#
# Math (per batch b):
#   xc = x^T                                  (D, L) channel-major
#   q  = Wq xc + bq                           (D, L)
#   offset path: conv(q grouped, Woff1) -> Woff2 -> tanh*K -> bilinear sample
#     positions, sample xc per group -> xs    (D, L)
#   k = Wk xs + bk ; v = Wv xs + bv + rel_bias
#   per head (64 ch):  sT = k_h^T q_h summed over L   (64x64)
#   attn = softmax_j(sT * D^-0.5) ; out_h = attn @ v_h ; y = out^T Wo^T + bo
#
# Key host-side folding: the whole offset conv collapses.  With
#   Weff[i,t] = sum_o Woff2[o] Woff1[o,i,t]        (256, 5)
#   wproj[g,t,:] = Weff[:,t]^T @ Wq[g*256:(g+1)*256, :]   -> Poff (D, 20)
# the pre-tanh offset signal is a 20-channel projection of x plus shifted
# sums -- no 85 GFLOP conv on device.
#
# Device pipeline per core (1 batch element per core, 8 cores):
#   pass 1: stream x chunks -> PE-transpose -> xc -> qT (token-major, spill to
#           DRAM) + off_pre (20, L) -> shifted-tap accumulate (DMA accum)
#   offset math: tanh -> positions -> floor/frac (via fmod) -> clamped int16
#           indices (16-wrapped via tiny PE transposes) + bilinear weights
#           (token-major via PE transposes)
#   pass 2: dma_gather rows of x from HBM (2 taps x 4 groups), bilinear
#           combine token-major, PE-transpose -> xs, K GEMM -> kT, V GEMM ->
#           v (+rel_bias, spill), score matmuls accumulate in PSUM
#   softmax: exp on ACT (no max-sub needed; logits ~ +-5), row sums via
#           matmul with ones, reciprocal
#   pass 3: out_h = E^T.T @ v_h scaled by 1/rowsum, final Wo GEMM -> y
#
# All GEMMs run with float32r (FP22 reads, full PE rate at N>=256).

import sys

if "/opt/trn_rl_repo" not in sys.path:
    sys.path.insert(0, "/opt/trn_rl_repo")

from contextlib import ExitStack

import numpy as np

import concourse.bass as bass
import concourse.tile as tile
from concourse import bacc
from concourse import mybir
from concourse.bass_utils import run_bass_kernel_spmd
from concourse.masks import make_identity

f32 = mybir.dt.float32
f32r = mybir.dt.float32r
bf16 = mybir.dt.bfloat16
i16 = mybir.dt.int16
Alu = mybir.AluOpType
Act = mybir.ActivationFunctionType

B, L, D = 8, 4096, 1024
H, G, KW = 16, 4, 5
DG, DH = D // G, D // H
NCORES = 8

C1 = 512            # pass-1 chunk (tokens)
NC1 = L // C1
C2 = 256            # pass-2 chunk
NC2 = L // C2
C3 = 512            # pass-3 chunk
NC3 = L // C3

S_NORM = float(L) / float(L + 3)   # position scale from grid_sample algebra
ATT_SCALE = 1.0 / float(D) ** 0.5


def build_program(has_bq: bool, has_bk: bool, has_bo: bool) -> bass.Bass:
    nc = bacc.Bacc("TRN2", debug=False)

    x_e = nc.declare_dram_parameter("x", [L, D], f32, isOutput=False)
    wq_e = nc.declare_dram_parameter("WqTs", [128, 8, D], f32r, isOutput=False)
    wk_e = nc.declare_dram_parameter("WkTs", [128, 8, D], f32r, isOutput=False)
    wv_e = nc.declare_dram_parameter("WvTs", [128, 8, D], f32r, isOutput=False)
    wo_e = nc.declare_dram_parameter("WoTs", [128, 8, D], f32r, isOutput=False)
    poff_e = nc.declare_dram_parameter("Poffs", [128, 8, 128], f32r, isOutput=False)
    ones_e = nc.declare_dram_parameter("ones2", [128, 2], f32r, isOutput=False)
    offb_e = nc.declare_dram_parameter("offbias", [20, 1], f32, isOutput=False)
    relb_e = nc.declare_dram_parameter("relb", [128, 8, L], bf16, isOutput=False)
    ios_e = nc.declare_dram_parameter("ios16", [4, L], f32, isOutput=False)
    c0_e = nc.declare_dram_parameter("c0vec", [128, 1], f32, isOutput=False)
    pad2_e = nc.declare_dram_parameter("pad2", [4, 2], f32, isOutput=False)
    if has_bq:
        bq_e = nc.declare_dram_parameter("bqrep", [128, D], f32, isOutput=False)
    if has_bk:
        bk_e = nc.declare_dram_parameter("bkrep", [128, D], f32, isOutput=False)
    if has_bo:
        bo_e = nc.declare_dram_parameter("borep", [128, D], f32, isOutput=False)
    y_e = nc.declare_dram_parameter("y", [L, D], f32, isOutput=True)

    qT_d = nc.dram_tensor("qT_scratch", [L, D], f32r)
    v_d = nc.dram_tensor("v_scratch", [128, 8, L], f32r)

    with tile.TileContext(nc) as tc, ExitStack() as ctx:
        singles = ctx.enter_context(tc.tile_pool(name="singles", bufs=1))
        wpool = ctx.enter_context(tc.tile_pool(name="wpool", bufs=2))
        ps_t = ctx.enter_context(tc.tile_pool(name="ps_t", bufs=2, space="PSUM"))
        ps_g = ctx.enter_context(tc.tile_pool(name="ps_g", bufs=2, space="PSUM"))
        ps_v = ctx.enter_context(tc.tile_pool(name="ps_v", bufs=2, space="PSUM"))
        ps_s = ctx.enter_context(tc.tile_pool(name="ps_s", bufs=1, space="PSUM"))

        ident = singles.tile([128, 128], f32, name="ident")
        make_identity(nc, ident)

        wq = wpool.tile([128, 8, D], f32r, tag="w", name="wq")
        nc.scalar.dma_start(out=wq, in_=wq_e[:])
        poffs = singles.tile([128, 8, 128], f32r, name="poffs")
        ones2 = singles.tile([128, 2], f32r, name="ones2")
        nc.sync.dma_start(out=ones2, in_=ones_e[:])
        nc.sync.dma_start(out=poffs, in_=poff_e[:])
        offb = singles.tile([20, 1], f32, name="offb")
        nc.sync.dma_start(out=offb, in_=offb_e[:])
        c0v = singles.tile([128, 1], f32, name="c0v")
        nc.sync.dma_start(out=c0v, in_=c0_e[:])
        if has_bq:
            bqr = singles.tile([128, D], f32, name="bqr")
            nc.sync.dma_start(out=bqr, in_=bq_e[:])
        if has_bk:
            bkr = singles.tile([128, D], f32, name="bkr")
            nc.sync.dma_start(out=bkr, in_=bk_e[:])
        if has_bo:
            bor = singles.tile([128, D], f32, name="bor")
            nc.sync.dma_start(out=bor, in_=bo_e[:])

        # pre-tanh offset accumulator over the 5 conv taps (pad 8 absorbs the
        # shifted accumulate windows; cols >= L are spillover junk)
        offraw = singles.tile([4, L + 8], f32, name="offraw")
        nc.vector.memset(offraw[:], 0.0)

        wtok = singles.tile([128, 32, 8], f32, name="wtok")
        idxw = singles.tile([128, NC2, 8, C2 // 16], i16, name="idxw")
        # separate PSUM banks for even/odd heads so each bank has exactly one
        # accumulation-group start/stop
        score_e = ps_s.tile([128, 512], f32, tag="score_e", name="score_e")
        score_o = ps_s.tile([128, 512], f32, tag="score_o", name="score_o")

        # ------------------------- PASS 1 -------------------------
        with tc.tile_pool(name="p1", bufs=2) as p1, \
             tc.tile_pool(name="p1o", bufs=2) as p1o:
            for c in range(NC1):
                l0 = c * C1
                xt = p1.tile([128, C1 // 128, D], f32, tag="xt", name="xt")
                nc.sync.dma_start(
                    out=xt[:],
                    in_=x_e[l0 : l0 + C1, :].rearrange("(ls p) d -> p ls d", p=128),
                )
                xc = p1.tile([128, 8, C1], f32r, tag="xc", name="xc")
                for dc in range(8):
                    pt = ps_t.tile([128, 512], f32, tag="t", name="pt")
                    for ls in range(C1 // 128):
                        nc.tensor.transpose(
                            pt[:, ls * 128 : (ls + 1) * 128],
                            xt[:, ls, dc * 128 : (dc + 1) * 128],
                            ident[:],
                        )
                    nc.scalar.activation(out=xc[:, dc, :], in_=pt[:], func=Act.Copy)
                qt = p1.tile([128, C1 // 128, D], f32r, tag="qt", name="qt")
                for ls in range(C1 // 128):
                    for oc in range(2):
                        pg = ps_g.tile([128, 512], f32, tag="g", name="pg")
                        for dc in range(8):
                            nc.tensor.matmul(
                                pg[:],
                                lhsT=xc[:, dc, ls * 128 : (ls + 1) * 128],
                                rhs=wq[:, dc, oc * 512 : (oc + 1) * 512],
                                start=(dc == 0),
                                stop=(dc == 7),
                            )
                        nc.vector.tensor_copy(
                            out=qt[:, ls, oc * 512 : (oc + 1) * 512], in_=pg[:]
                        )
                if has_bq:
                    for ls in range(C1 // 128):
                        nc.vector.tensor_tensor(
                            out=qt[:, ls, :], in0=qt[:, ls, :], in1=bqr[:], op=Alu.add
                        )
                nc.sync.dma_start(
                    out=qT_d[l0 : l0 + C1, :].rearrange("(ls p) o -> p ls o", p=128),
                    in_=qt[:],
                )
                # off_pre: (20, C1) = Poff^T @ xc_chunk
                po = ps_v.tile([128, 512], f32, tag="v", name="po")
                for dc in range(8):
                    nc.tensor.matmul(
                        po[:, :C1],
                        lhsT=poffs[:, dc, :],
                        rhs=xc[:, dc, :],
                        start=(dc == 0),
                        stop=(dc == 7),
                    )
                offsb = p1o.tile([20, C1], f32, tag="offsb", name="offsb")
                nc.scalar.activation(
                    out=offsb[:], in_=po[:20, :C1], func=Act.Identity,
                    bias=offb[:], scale=1.0,
                )
                # shifted tap accumulate: off_pre[4t+g, l'] adds into
                # offraw[g, l' + 4 - t]
                for t in range(KW):
                    nc.gpsimd.dma_start(
                        out=offraw[0:4, l0 + 4 - t : l0 + 4 - t + C1],
                        in_=offsb[4 * t : 4 * t + 4, :],
                        accum_op=Alu.add,
                    )

        wk = wpool.tile([128, 8, D], f32r, tag="w", name="wk")
        nc.scalar.dma_start(out=wk, in_=wk_e[:])
        wv = wpool.tile([128, 8, D], f32r, tag="w", name="wv")
        nc.scalar.dma_start(out=wv, in_=wv_e[:])

        # ------------------------- OFFSET MATH -------------------------
        # Fold [4, L] -> [128, 128] (partition = g*32 + l//128) so the
        # elementwise chain uses all DVE lanes; fold/unfold are cheap DMAs.
        # arri rows: 0-3 = i0 clamped per group, 4-7 = i1 clamped
        # arrw rows: 0-3 = w0' (=(1-frac)*valid0), 4-7 = w1' (=frac*valid1)
        with tc.tile_pool(name="om", bufs=1) as om, \
             tc.tile_pool(name="omb", bufs=1) as omb:
            arri = omb.tile([8, L], f32, name="arri")
            arrw = omb.tile([8, L], f32, name="arrw")
            zf = om.tile([128, 128], f32, tag="zf", name="zf")
            nc.sync.dma_start(
                out=zf[:],
                in_=offraw[0:4, 0:L].rearrange("g (lb li) -> g lb li", li=128),
            )
            iosb = om.tile([128, 128], f32, tag="iosb", name="iosb")
            nc.sync.dma_start(out=iosb[:], in_=ios_e[:].rearrange(
                "g (lb li) -> g lb li", li=128))
            nc.vector.tensor_scalar(zf[:], zf[:], c0v[:, 0:1], None, Alu.add)
            # reference: offset for l<2 is exactly boff2 (the concat pad);
            # l in {0,1} sits at partitions g*32, cols 0:2
            for g in range(G):
                nc.sync.dma_start(out=zf[32 * g : 32 * g + 1, 0:2],
                                  in_=pad2_e[g : g + 1, :])
            th = om.tile([128, 128], f32, tag="th", name="th")
            nc.scalar.activation(out=th[:], in_=zf[:], func=Act.Tanh)
            posp = om.tile([128, 128], f32, tag="posp", name="posp")
            # posp = 16 + pos = KW*tanh(z)*S + (iota*S - 0.5 + 16)
            nc.vector.tensor_scalar(posp[:], th[:], float(KW) * S_NORM, None,
                                    Alu.mult)
            nc.vector.tensor_tensor(out=posp[:], in0=posp[:], in1=iosb[:],
                                    op=Alu.add)
            i16t = om.tile([128, 128], i16, tag="i16t", name="i16t")
            nc.vector.tensor_copy(out=i16t[:], in_=posp[:])
            flo = om.tile([128, 128], f32, tag="flo", name="flo")
            nc.vector.tensor_copy(out=flo[:], in_=i16t[:])
            # rounding-mode-agnostic floor: flo -= (flo > posp)
            corr = om.tile([128, 128], f32, tag="corr", name="corr")
            nc.vector.tensor_tensor(out=corr[:], in0=flo[:], in1=posp[:],
                                    op=Alu.subtract)
            nc.vector.tensor_scalar(corr[:], corr[:], float(2 ** 25), 1.0,
                                    Alu.mult, Alu.min)
            nc.vector.tensor_scalar(corr[:], corr[:], 0.0, None, Alu.max)
            nc.vector.tensor_tensor(out=flo[:], in0=flo[:], in1=corr[:],
                                    op=Alu.subtract)
            w1 = om.tile([128, 128], f32, tag="w1", name="w1")
            nc.vector.tensor_tensor(out=w1[:], in0=posp[:], in1=flo[:],
                                    op=Alu.subtract)
            i0f = om.tile([128, 128], f32, tag="i0f", name="i0f")
            nc.vector.tensor_scalar(i0f[:], flo[:], -16.0, None, Alu.add)
            # valid masks from clips (integer-valued i0f):
            v0 = om.tile([128, 128], f32, tag="v0", name="v0")
            nc.vector.tensor_scalar(v0[:], i0f[:], 1.0, 1.0, Alu.add, Alu.min)
            nc.vector.tensor_scalar(v0[:], v0[:], 0.0, None, Alu.max)
            vtmp = om.tile([128, 128], f32, tag="vtmp", name="vtmp")
            nc.vector.tensor_scalar(vtmp[:], i0f[:], -1.0, 4096.0, Alu.mult,
                                    Alu.add)
            nc.vector.tensor_scalar(vtmp[:], vtmp[:], 1.0, 0.0, Alu.min, Alu.max)
            nc.vector.tensor_tensor(out=v0[:], in0=v0[:], in1=vtmp[:],
                                    op=Alu.mult)
            u0 = om.tile([128, 128], f32, tag="u0", name="u0")
            nc.vector.tensor_scalar(u0[:], i0f[:], 2.0, 1.0, Alu.add, Alu.min)
            nc.vector.tensor_scalar(u0[:], u0[:], 0.0, None, Alu.max)
            nc.vector.tensor_scalar(vtmp[:], i0f[:], -1.0, 4095.0, Alu.mult,
                                    Alu.add)
            nc.vector.tensor_scalar(vtmp[:], vtmp[:], 1.0, 0.0, Alu.min, Alu.max)
            nc.vector.tensor_tensor(out=u0[:], in0=u0[:], in1=vtmp[:],
                                    op=Alu.mult)
            w1p = om.tile([128, 128], f32, tag="w1p", name="w1p")
            nc.vector.tensor_tensor(out=w1p[:], in0=w1[:], in1=u0[:],
                                    op=Alu.mult)
            w0p = om.tile([128, 128], f32, tag="w0p", name="w0p")
            nc.vector.tensor_scalar(w0p[:], w1[:], -1.0, 1.0, Alu.mult, Alu.add)
            nc.vector.tensor_tensor(out=w0p[:], in0=w0p[:], in1=v0[:],
                                    op=Alu.mult)
            i0c = om.tile([128, 128], f32, tag="i0c", name="i0c")
            nc.vector.tensor_scalar(i0c[:], i0f[:], 0.0, 4095.0, Alu.max,
                                    Alu.min)
            i1c = om.tile([128, 128], f32, tag="i1c", name="i1c")
            nc.vector.tensor_scalar(i1c[:], i0f[:], 1.0, 0.0, Alu.add, Alu.max)
            nc.vector.tensor_scalar(i1c[:], i1c[:], 4095.0, None, Alu.min)
            # unfold into 8-row stacks for the PE wrap/weight transposes
            for srcf, dst, r0 in ((i0c, arri, 0), (i1c, arri, 4),
                                  (w0p, arrw, 0), (w1p, arrw, 4)):
                nc.sync.dma_start(
                    out=dst[r0 : r0 + 4, :].rearrange(
                        "g (lb li) -> g lb li", li=128),
                    in_=srcf[:],
                )

            # 16-wrapped int16 index layout for dma_gather: idx set r=(tap*4+g),
            # wrapped[p, k] = value at token 16k+p
            for pt4 in range(4):
                pw = ps_t.tile([16, 512], f32, tag="t", name="pw")
                for kk in range(64):
                    k = pt4 * 64 + kk
                    nc.tensor.transpose(
                        pw[:, kk * 8 : (kk + 1) * 8],
                        arri[:, k * 16 : (k + 1) * 16],
                        ident[0:8, 0:8],
                    )
                # chunks of C2 tokens = 16 k-blocks; 4 chunks per psum tile
                nc.vector.tensor_copy(
                    out=idxw[0:16, pt4 * 4 : (pt4 + 1) * 4, :, :],
                    in_=pw[:].rearrange("p (c kk st) -> p c st kk", c=4, kk=16,
                                        st=8),
                )
            for r in range(1, 8):
                nc.sync.dma_start(out=idxw[16 * r : 16 * (r + 1)], in_=idxw[0:16])

            # token-major bilinear weights: wtok[p, B, r] = arrw[r, 128B + p]
            pww = ps_t.tile([128, 512], f32, tag="t", name="pww")
            for b in range(32):
                nc.tensor.transpose(
                    pww[:, b * 8 : (b + 1) * 8],
                    arrw[:, b * 128 : (b + 1) * 128],
                    ident[0:8, 0:8],
                )
            nc.vector.tensor_copy(out=wtok[:], in_=pww[:, 0:256])

        # ------------------------- PASS 2 -------------------------
        nidx_reg = nc.gpsimd.to_reg(C2)
        with tc.tile_pool(name="p2", bufs=2) as p2, \
             tc.tile_pool(name="p2b", bufs=1) as p2b, \
             tc.tile_pool(name="p2g", bufs=4) as p2g:
            for c in range(NC2):
                l0 = c * C2
                qt2 = p2.tile([128, C2 // 128, D], f32r, tag="qt2", name="qt2")
                nc.sync.dma_start(
                    out=qt2[:],
                    in_=qT_d[l0 : l0 + C2, :].rearrange("(ls p) o -> p ls o", p=128),
                )
                relb = p2.tile([128, 8, C2], bf16, tag="relb", name="relb")
                nc.gpsimd.dma_start(out=relb[:], in_=relb_e[:, :, l0 : l0 + C2])
                xsT = p2.tile([128, C2 // 128, D], f32, tag="xsT", name="xsT")
                for g in range(G):
                    ga = p2g.tile([128, C2 // 128, 256], f32, tag="gth", name="ga")
                    nc.gpsimd.dma_gather(
                        out_ap=ga[:],
                        in_ap=x_e[:, g * 256 : (g + 1) * 256],
                        idxs_ap=idxw[:, c, g, :],
                        num_idxs=C2,
                        num_idxs_reg=nidx_reg,
                        elem_size=256,
                        elem_step=D,
                    )
                    gb = p2g.tile([128, C2 // 128, 256], f32, tag="gth", name="gb")
                    nc.gpsimd.dma_gather(
                        out_ap=gb[:],
                        in_ap=x_e[:, g * 256 : (g + 1) * 256],
                        idxs_ap=idxw[:, c, 4 + g, :],
                        num_idxs=C2,
                        num_idxs_reg=nidx_reg,
                        elem_size=256,
                        elem_step=D,
                    )
                    nb = C2 // 128
                    wa = wtok[:, nb * c : nb * (c + 1), g, None].to_broadcast(
                        (128, nb, 256))
                    wb = wtok[:, nb * c : nb * (c + 1), 4 + g, None].to_broadcast(
                        (128, nb, 256))
                    nc.vector.tensor_tensor(out=ga[:], in0=ga[:], in1=wa, op=Alu.mult)
                    nc.vector.tensor_tensor(out=gb[:], in0=gb[:], in1=wb, op=Alu.mult)
                    nc.vector.tensor_tensor(
                        out=xsT[:, :, g * 256 : (g + 1) * 256], in0=ga[:], in1=gb[:],
                        op=Alu.add,
                    )
                xs = p2b.tile([128, 8, C2], f32r, tag="xs", name="xs")
                for dc in range(8):
                    pt = ps_t.tile([128, 512], f32, tag="t", name="pt2")
                    for ls in range(C2 // 128):
                        nc.tensor.transpose(
                            pt[:, ls * 128 : (ls + 1) * 128],
                            xsT[:, ls, dc * 128 : (dc + 1) * 128],
                            ident[:],
                        )
                    nc.scalar.activation(out=xs[:, dc, :], in_=pt[:, 0:C2],
                                         func=Act.Copy)
                kt = p2b.tile([128, C2 // 128, D], f32r, tag="kt", name="kt")
                for ls in range(C2 // 128):
                    for oc in range(2):
                        pg = ps_g.tile([128, 512], f32, tag="g", name="pg2")
                        for dc in range(8):
                            nc.tensor.matmul(
                                pg[:],
                                lhsT=xs[:, dc, ls * 128 : (ls + 1) * 128],
                                rhs=wk[:, dc, oc * 512 : (oc + 1) * 512],
                                start=(dc == 0),
                                stop=(dc == 7),
                            )
                        nc.scalar.activation(
                            out=kt[:, ls, oc * 512 : (oc + 1) * 512], in_=pg[:],
                            func=Act.Copy,
                        )
                if has_bk:
                    for ls in range(C2 // 128):
                        nc.vector.tensor_tensor(
                            out=kt[:, ls, :], in0=kt[:, ls, :], in1=bkr[:], op=Alu.add
                        )
                vsb = p2.tile([128, 8, C2], f32r, tag="vsb", name="vsb")
                for dc in range(8):
                    pv = ps_v.tile([128, 512], f32, tag="v", name="pv")
                    for ds in range(8):
                        nc.tensor.matmul(
                            pv[:, :C2],
                            lhsT=wv[:, ds, dc * 128 : (dc + 1) * 128],
                            rhs=xs[:, ds, :],
                            start=(ds == 0),
                            stop=(ds == 7),
                        )
                    nc.vector.tensor_tensor(
                        out=vsb[:, dc, :], in0=pv[:, :C2], in1=relb[:, dc, :],
                        op=Alu.add,
                    )
                nc.sync.dma_start(out=v_d[:, :, l0 : l0 + C2], in_=vsb[:])
                # scores: sT[j, i] accumulated across all chunks in one PSUM bank
                # head-pair blocks: pair p covers channels [128p, 128p+128);
                # the [128,128] block has the two per-head scores on its
                # diagonal 64-blocks (off-diagonals are junk, zeroed later)
                for ls in range(C2 // 128):
                    for p in range(8):
                        sbank = score_e if p < 4 else score_o
                        pc = (p % 4) * 128
                        nc.tensor.matmul(
                            sbank[:, pc : pc + 128],
                            lhsT=kt[:, ls, 128 * p : 128 * p + 128],
                            rhs=qt2[:, ls, 128 * p : 128 * p + 128],
                            start=(c == 0 and ls == 0 and p % 4 == 0),
                            stop=(c == NC2 - 1 and ls == C2 // 128 - 1
                                  and p % 4 == 3),
                        )

        wo = wpool.tile([128, 8, D], f32r, tag="w", name="wo")
        nc.scalar.dma_start(out=wo, in_=wo_e[:])

        # ------------------------- SOFTMAX -------------------------
        esb = singles.tile([128, 1024], f32r, name="esb")
        nc.vector.memset(esb[:].bitcast(f32), 0.0)
        for h in range(H):
            p = h // 2
            sbank = score_e if p < 4 else score_o
            pc = (p % 4) * 128
            r0 = 64 * (h % 2)
            nc.scalar.activation(
                out=esb[r0 : r0 + 64, 128 * p + r0 : 128 * p + r0 + 64],
                in_=sbank[r0 : r0 + 64, pc + r0 : pc + r0 + 64],
                func=Act.Exp, scale=ATT_SCALE,
            )
        rs_ps = ps_g.tile([128, 16], f32, tag="g", name="rs_ps")
        for p in range(8):
            nc.tensor.matmul(
                rs_ps[:, 2 * p : 2 * p + 2],
                lhsT=esb[:, 128 * p : 128 * p + 128],
                rhs=ones2[:],
                start=True,
                stop=True,
            )
        rsi = singles.tile([128, 16], f32, name="rsi")
        nc.vector.reciprocal(out=rsi[:], in_=rs_ps[:])

        # ------------------------- PASS 3 -------------------------
        with tc.tile_pool(name="p3", bufs=2) as p3, tc.tile_pool(name="p3b", bufs=1) as p3b:
            for c in range(NC3):
                l0 = c * C3
                vs3 = p3.tile([128, 8, C3], f32r, tag="v3", name="vs3")
                nc.sync.dma_start(out=vs3[:], in_=v_d[:, :, l0 : l0 + C3])
                osb = p3b.tile([128, 8, C3], f32r, tag="o3", name="osb")
                for p in range(8):
                    po3 = ps_v.tile([128, 512], f32, tag="v", name="po3")
                    nc.tensor.matmul(
                        po3[:],
                        lhsT=esb[:, 128 * p : 128 * p + 128],
                        rhs=vs3[:, p, :],
                        start=True,
                        stop=True,
                    )
                    nc.scalar.activation(
                        out=osb[:, p, :], in_=po3[:], func=Act.Identity,
                        bias=0.0, scale=rsi[:, 2 * p : 2 * p + 1],
                    )
                yt = p3.tile([128, C3 // 128, D], f32, tag="yt", name="yt")
                for ls in range(C3 // 128):
                    for oc in range(2):
                        pg = ps_g.tile([128, 512], f32, tag="g", name="pg3")
                        for dc in range(8):
                            nc.tensor.matmul(
                                pg[:],
                                lhsT=osb[:, dc, ls * 128 : (ls + 1) * 128],
                                rhs=wo[:, dc, oc * 512 : (oc + 1) * 512],
                                start=(dc == 0),
                                stop=(dc == 7),
                            )
                        nc.vector.tensor_copy(
                            out=yt[:, ls, oc * 512 : (oc + 1) * 512], in_=pg[:]
                        )
                if has_bo:
                    for ls in range(C3 // 128):
                        nc.vector.tensor_tensor(
                            out=yt[:, ls, :], in0=yt[:, ls, :], in1=bor[:], op=Alu.add
                        )
                nc.sync.dma_start(
                    out=y_e[l0 : l0 + C3, :].rearrange("(ls p) o -> p ls o", p=128),
                    in_=yt[:],
                )

    nc.compile()
    return nc


_prog_cache: dict = {}


def get_program(has_bq: bool, has_bk: bool, has_bo: bool) -> bass.Bass:
    key = (has_bq, has_bk, has_bo)
    if key not in _prog_cache:
        _prog_cache[key] = build_program(*key)
    return _prog_cache[key]


def make_in_maps(inputs: dict) -> tuple[list[dict], tuple]:
    import ml_dtypes

    x = np.ascontiguousarray(np.asarray(inputs["x"], np.float32))
    Wq = np.asarray(inputs["Wq"], np.float32)
    bq = np.asarray(inputs["bq"], np.float32)
    Wk = np.asarray(inputs["Wk"], np.float32)
    bk = np.asarray(inputs["bk"], np.float32)
    Wv = np.asarray(inputs["Wv"], np.float32)
    bv = np.asarray(inputs["bv"], np.float32)
    Wo = np.asarray(inputs["Wo"], np.float32)
    bo = np.asarray(inputs["bo"], np.float32)
    Woff1 = np.asarray(inputs["Woff1"], np.float32)
    boff1 = np.asarray(inputs["boff1"], np.float32)
    Woff2 = np.asarray(inputs["Woff2"], np.float32)
    boff2 = np.asarray(inputs["boff2"], np.float32)
    rel_bias = np.asarray(inputs["rel_bias"], np.float32)

    def wts(w):  # (D, D) weight -> lhsT layout [128, 8, D]: [p, dc, o] = w[o, dc*128+p]
        return np.ascontiguousarray(
            w.T.reshape(8, 128, D).transpose(1, 0, 2).astype(np.float32)
        )

    # offset-path folding
    Weff = np.einsum("o,oit->it", Woff2[0].astype(np.float64),
                     Woff1.astype(np.float64))            # (256, 5)
    Poff = np.zeros((D, 128), np.float64)                 # padded to M=128 for fp32r
    offbias = np.zeros((20, 1), np.float64)
    for t in range(KW):
        for g in range(G):
            r = 4 * t + g
            Poff[:, r] = Weff[:, t] @ Wq[g * DG : (g + 1) * DG, :].astype(np.float64)
            offbias[r, 0] = Weff[:, t] @ bq[g * DG : (g + 1) * DG].astype(np.float64)
    c0 = float(Woff2[0].astype(np.float64) @ boff1.astype(np.float64)
               + np.float64(boff2[0]))
    Poffs = np.ascontiguousarray(
        Poff.reshape(8, 128, 128).transpose(1, 0, 2).astype(np.float32)
    )

    rel_eff = rel_bias[0] + bv[:, None]                   # (D, L)
    relb = np.ascontiguousarray(
        rel_eff.reshape(8, 128, L).transpose(1, 0, 2).astype(ml_dtypes.bfloat16)
    )

    ios16 = np.tile(
        (np.arange(L, dtype=np.float64) * S_NORM - 0.5 + 16.0)[None, :], (4, 1)
    ).astype(np.float32)
    c0vec = np.full((128, 1), c0, np.float32)
    pad2 = np.full((4, 2), float(boff2[0]), np.float32)

    has_bq = bool(np.any(bq != 0.0))
    has_bk = bool(np.any(bk != 0.0))
    has_bo = bool(np.any(bo != 0.0))

    shared = {
        "WqTs": wts(Wq),
        "WkTs": wts(Wk),
        "WvTs": wts(Wv),
        "WoTs": wts(Wo),
        "Poffs": Poffs,
        "offbias": offbias.astype(np.float32),
        "relb": relb,
        "ios16": ios16,
        "c0vec": c0vec,
        "pad2": pad2,
        "ones2": np.ones((128, 2), np.float32),
    }
    if has_bq:
        shared["bqrep"] = np.ascontiguousarray(np.tile(bq[None, :], (128, 1)))
    if has_bk:
        shared["bkrep"] = np.ascontiguousarray(np.tile(bk[None, :], (128, 1)))
    if has_bo:
        shared["borep"] = np.ascontiguousarray(np.tile(bo[None, :], (128, 1)))

    in_maps = [dict(shared, x=np.ascontiguousarray(x[i])) for i in range(NCORES)]
    return in_maps, (has_bq, has_bk, has_bo)


def kernel_run(inputs: dict, trace: bool = False):
    in_maps, flags = make_in_maps(inputs)
    nc = get_program(*flags)
    res = run_bass_kernel_spmd(nc, in_maps, list(range(NCORES)), trace=trace)
    y = np.stack([res.results[i]["y"] for i in range(NCORES)], axis=0)
    return y, res


def kernel(**inputs) -> np.ndarray:
    y, _ = kernel_run(inputs, trace=False)
    return y


# revision 16
# speedup vs baseline: 1.2265x; 1.0576x over previous
# Trainium2 Bass kernel for nn_CrossDeformAttn (deformable cross attention).
#
# Math (per batch b):
#   xc = x^T                                  (D, L) channel-major
#   q  = Wq xc + bq                           (D, L)
#   offset path: conv(q grouped, Woff1) -> Woff2 -> tanh*K -> bilinear sample
#     positions, sample xc per group -> xs    (D, L)
#   k = Wk xs + bk ; v = Wv xs + bv + rel_bias
#   per head (64 ch):  sT = k_h^T q_h summed over L   (64x64)
#   attn = softmax_j(sT * D^-0.5) ; out_h = attn @ v_h ; y = out^T Wo^T + bo
#
# Key host-side folding: the whole offset conv collapses.  With
#   Weff[i,t] = sum_o Woff2[o] Woff1[o,i,t]        (256, 5)
#   wproj[g,t,:] = Weff[:,t]^T @ Wq[g*256:(g+1)*256, :]   -> Poff (D, 20)
# the pre-tanh offset signal is a 20-channel projection of x plus shifted
# sums -- no 85 GFLOP conv on device.
#
# Device pipeline per core (1 batch element per core, 8 cores):
#   pass 1: stream x chunks -> PE-transpose -> xc -> qT (token-major, spill to
#           DRAM) + off_pre (20, L) -> shifted-tap accumulate (DMA accum)
#   offset math: tanh -> positions -> floor/frac (via fmod) -> clamped int16
#           indices (16-wrapped via tiny PE transposes) + bilinear weights
#           (token-major via PE transposes)
#   pass 2: dma_gather rows of x from HBM (2 taps x 4 groups), bilinear
#           combine token-major, PE-transpose -> xs, K GEMM -> kT, V GEMM ->
#           v (+rel_bias, spill), score matmuls accumulate in PSUM
#   softmax: exp on ACT (no max-sub needed; logits ~ +-5), row sums via
#           matmul with ones, reciprocal
#   pass 3: out_h = E^T.T @ v_h scaled by 1/rowsum, final Wo GEMM -> y
#
# All GEMMs run with float32r (FP22 reads, full PE rate at N>=256).

import sys

if "/opt/trn_rl_repo" not in sys.path:
    sys.path.insert(0, "/opt/trn_rl_repo")

from contextlib import ExitStack

import numpy as np

import concourse.bass as bass
import concourse.tile as tile
from concourse import bacc
from concourse import mybir
from concourse.bass_utils import run_bass_kernel_spmd
from concourse.masks import make_identity

f32 = mybir.dt.float32
f32r = mybir.dt.float32r
bf16 = mybir.dt.bfloat16
i16 = mybir.dt.int16
Alu = mybir.AluOpType
Act = mybir.ActivationFunctionType

B, L, D = 8, 4096, 1024
H, G, KW = 16, 4, 5
DG, DH = D // G, D // H
NCORES = 8

C1 = 512            # pass-1 chunk (tokens)
NC1 = L // C1
C2 = 256            # pass-2 chunk
NC2 = L // C2
C3 = 512            # pass-3 chunk
NC3 = L // C3

S_NORM = float(L) / float(L + 3)   # position scale from grid_sample algebra
ATT_SCALE = 1.0 / float(D) ** 0.5


def build_program(has_bq: bool, has_bk: bool, has_bo: bool) -> bass.Bass:
    nc = bacc.Bacc("TRN2", debug=False)

    x_e = nc.declare_dram_parameter("x", [L, D], f32, isOutput=False)
    wq_e = nc.declare_dram_parameter("WqTs", [128, 8, D], f32r, isOutput=False)
    wk_e = nc.declare_dram_parameter("WkTs", [128, 8, D], f32r, isOutput=False)
    wv_e = nc.declare_dram_parameter("WvTs", [128, 8, D], f32r, isOutput=False)
    wo_e = nc.declare_dram_parameter("WoTs", [128, 8, D], f32r, isOutput=False)
    poff_e = nc.declare_dram_parameter("Poffs", [128, 8, 128], f32r, isOutput=False)
    ones_e = nc.declare_dram_parameter("ones2", [128, 2], f32r, isOutput=False)
    offb_e = nc.declare_dram_parameter("offbias", [20, 1], f32, isOutput=False)
    relb_e = nc.declare_dram_parameter("relb", [128, 8, L], bf16, isOutput=False)
    ios_e = nc.declare_dram_parameter("ios16", [4, L], f32, isOutput=False)
    c0_e = nc.declare_dram_parameter("c0vec", [128, 1], f32, isOutput=False)
    pad2_e = nc.declare_dram_parameter("pad2", [4, 2], f32, isOutput=False)
    if has_bq:
        bq_e = nc.declare_dram_parameter("bqrep", [128, D], f32, isOutput=False)
    if has_bk:
        bk_e = nc.declare_dram_parameter("bkrep", [128, D], f32, isOutput=False)
    if has_bo:
        bo_e = nc.declare_dram_parameter("borep", [128, D], f32, isOutput=False)
    y_e = nc.declare_dram_parameter("y", [L, D], f32, isOutput=True)

    qT_d = nc.dram_tensor("qT_scratch", [L, D], bf16)
    v_d = nc.dram_tensor("v_scratch", [128, 8, L], f32r)

    with tile.TileContext(nc) as tc, ExitStack() as ctx:
        singles = ctx.enter_context(tc.tile_pool(name="singles", bufs=1))
        wpool = ctx.enter_context(tc.tile_pool(name="wpool", bufs=2))
        ps_t = ctx.enter_context(tc.tile_pool(name="ps_t", bufs=2, space="PSUM"))
        ps_g = ctx.enter_context(tc.tile_pool(name="ps_g", bufs=2, space="PSUM"))
        ps_v = ctx.enter_context(tc.tile_pool(name="ps_v", bufs=2, space="PSUM"))
        ps_s = ctx.enter_context(tc.tile_pool(name="ps_s", bufs=1, space="PSUM"))

        ident = singles.tile([128, 128], f32, name="ident")
        make_identity(nc, ident)

        wq = wpool.tile([128, 8, D], f32r, tag="w", name="wq")
        nc.scalar.dma_start(out=wq, in_=wq_e[:])
        poffs = singles.tile([128, 8, 128], f32r, name="poffs")
        ones2 = singles.tile([128, 2], f32r, name="ones2")
        nc.sync.dma_start(out=ones2, in_=ones_e[:])
        nc.sync.dma_start(out=poffs, in_=poff_e[:])
        offb = singles.tile([20, 1], f32, name="offb")
        nc.sync.dma_start(out=offb, in_=offb_e[:])
        c0v = singles.tile([128, 1], f32, name="c0v")
        nc.sync.dma_start(out=c0v, in_=c0_e[:])
        if has_bq:
            bqr = singles.tile([128, D], f32, name="bqr")
            nc.sync.dma_start(out=bqr, in_=bq_e[:])
        if has_bk:
            bkr = singles.tile([128, D], f32, name="bkr")
            nc.sync.dma_start(out=bkr, in_=bk_e[:])
        if has_bo:
            bor = singles.tile([128, D], f32, name="bor")
            nc.sync.dma_start(out=bor, in_=bo_e[:])

        # pre-tanh offset accumulator over the 5 conv taps (pad 8 absorbs the
        # shifted accumulate windows; cols >= L are spillover junk)
        offraw = singles.tile([4, L + 8], f32, name="offraw")
        nc.vector.memset(offraw[:], 0.0)

        wtok = singles.tile([128, 32, 8], f32, name="wtok")
        idxw = singles.tile([128, NC2, 8, C2 // 16], i16, name="idxw")
        # separate PSUM banks for even/odd heads so each bank has exactly one
        # accumulation-group start/stop
        score_e = ps_s.tile([128, 512], f32, tag="score_e", name="score_e")
        score_o = ps_s.tile([128, 512], f32, tag="score_o", name="score_o")

        # ------------------------- PASS 1 -------------------------
        with tc.tile_pool(name="p1", bufs=2) as p1, \
             tc.tile_pool(name="p1o", bufs=2) as p1o:
            for c in range(NC1):
                l0 = c * C1
                xt = p1.tile([128, C1 // 128, D], f32, tag="xt", name="xt")
                nc.sync.dma_start(
                    out=xt[:],
                    in_=x_e[l0 : l0 + C1, :].rearrange("(ls p) d -> p ls d", p=128),
                )
                xc = p1.tile([128, 8, C1], f32r, tag="xc", name="xc")
                for dc in range(8):
                    pt = ps_t.tile([128, 512], f32, tag="t", name="pt")
                    for ls in range(C1 // 128):
                        nc.tensor.transpose(
                            pt[:, ls * 128 : (ls + 1) * 128],
                            xt[:, ls, dc * 128 : (dc + 1) * 128],
                            ident[:],
                        )
                    nc.scalar.activation(out=xc[:, dc, :], in_=pt[:], func=Act.Copy)
                qt = p1.tile([128, C1 // 128, D], bf16, tag="qt", name="qt")
                for ls in range(C1 // 128):
                    for oc in range(2):
                        pg = ps_g.tile([128, 512], f32, tag="g", name="pg")
                        for dc in range(8):
                            nc.tensor.matmul(
                                pg[:],
                                lhsT=xc[:, dc, ls * 128 : (ls + 1) * 128],
                                rhs=wq[:, dc, oc * 512 : (oc + 1) * 512],
                                start=(dc == 0),
                                stop=(dc == 7),
                            )
                        nc.vector.tensor_copy(
                            out=qt[:, ls, oc * 512 : (oc + 1) * 512], in_=pg[:]
                        )
                if has_bq:
                    for ls in range(C1 // 128):
                        nc.vector.tensor_tensor(
                            out=qt[:, ls, :], in0=qt[:, ls, :], in1=bqr[:], op=Alu.add
                        )
                nc.sync.dma_start(
                    out=qT_d[l0 : l0 + C1, :].rearrange("(ls p) o -> p ls o", p=128),
                    in_=qt[:],
                )
                # off_pre: (20, C1) = Poff^T @ xc_chunk
                po = ps_v.tile([128, 512], f32, tag="v", name="po")
                for dc in range(8):
                    nc.tensor.matmul(
                        po[:, :C1],
                        lhsT=poffs[:, dc, :],
                        rhs=xc[:, dc, :],
                        start=(dc == 0),
                        stop=(dc == 7),
                    )
                offsb = p1o.tile([20, C1], f32, tag="offsb", name="offsb")
                nc.scalar.activation(
                    out=offsb[:], in_=po[:20, :C1], func=Act.Identity,
                    bias=offb[:], scale=1.0,
                )
                # shifted tap accumulate: off_pre[4t+g, l'] adds into
                # offraw[g, l' + 4 - t]
                for t in range(KW):
                    nc.gpsimd.dma_start(
                        out=offraw[0:4, l0 + 4 - t : l0 + 4 - t + C1],
                        in_=offsb[4 * t : 4 * t + 4, :],
                        accum_op=Alu.add,
                    )

        wk = wpool.tile([128, 8, D], f32r, tag="w", name="wk")
        nc.scalar.dma_start(out=wk, in_=wk_e[:])
        wv = wpool.tile([128, 8, D], f32r, tag="w", name="wv")
        nc.scalar.dma_start(out=wv, in_=wv_e[:])

        # ------------------------- OFFSET MATH -------------------------
        # Fold [4, L] -> [128, 128] (partition = g*32 + l//128) so the
        # elementwise chain uses all DVE lanes; fold/unfold are cheap DMAs.
        # arri rows: 0-3 = i0 clamped per group, 4-7 = i1 clamped
        # arrw rows: 0-3 = w0' (=(1-frac)*valid0), 4-7 = w1' (=frac*valid1)
        with tc.tile_pool(name="om", bufs=1) as om, \
             tc.tile_pool(name="omb", bufs=1) as omb:
            arri = omb.tile([8, L], f32, name="arri")
            arrw = omb.tile([8, L], f32, name="arrw")
            zf = om.tile([128, 128], f32, tag="zf", name="zf")
            nc.sync.dma_start(
                out=zf[:],
                in_=offraw[0:4, 0:L].rearrange("g (lb li) -> g lb li", li=128),
            )
            iosb = om.tile([128, 128], f32, tag="iosb", name="iosb")
            nc.sync.dma_start(out=iosb[:], in_=ios_e[:].rearrange(
                "g (lb li) -> g lb li", li=128))
            nc.vector.tensor_scalar(zf[:], zf[:], c0v[:, 0:1], None, Alu.add)
            # reference: offset for l<2 is exactly boff2 (the concat pad);
            # l in {0,1} sits at partitions g*32, cols 0:2
            for g in range(G):
                nc.sync.dma_start(out=zf[32 * g : 32 * g + 1, 0:2],
                                  in_=pad2_e[g : g + 1, :])
            th = om.tile([128, 128], f32, tag="th", name="th")
            nc.scalar.activation(out=th[:], in_=zf[:], func=Act.Tanh)
            posp = om.tile([128, 128], f32, tag="posp", name="posp")
            # posp = 16 + pos = KW*tanh(z)*S + (iota*S - 0.5 + 16)
            nc.vector.tensor_scalar(posp[:], th[:], float(KW) * S_NORM, None,
                                    Alu.mult)
            nc.vector.tensor_tensor(out=posp[:], in0=posp[:], in1=iosb[:],
                                    op=Alu.add)
            i16t = om.tile([128, 128], i16, tag="i16t", name="i16t")
            nc.vector.tensor_copy(out=i16t[:], in_=posp[:])
            flo = om.tile([128, 128], f32, tag="flo", name="flo")
            nc.vector.tensor_copy(out=flo[:], in_=i16t[:])
            # rounding-mode-agnostic floor: flo -= (flo > posp)
            corr = om.tile([128, 128], f32, tag="corr", name="corr")
            nc.vector.tensor_tensor(out=corr[:], in0=flo[:], in1=posp[:],
                                    op=Alu.subtract)
            nc.vector.tensor_scalar(corr[:], corr[:], float(2 ** 25), 1.0,
                                    Alu.mult, Alu.min)
            nc.vector.tensor_scalar(corr[:], corr[:], 0.0, None, Alu.max)
            nc.vector.tensor_tensor(out=flo[:], in0=flo[:], in1=corr[:],
                                    op=Alu.subtract)
            w1 = om.tile([128, 128], f32, tag="w1", name="w1")
            nc.vector.tensor_tensor(out=w1[:], in0=posp[:], in1=flo[:],
                                    op=Alu.subtract)
            i0f = om.tile([128, 128], f32, tag="i0f", name="i0f")
            nc.vector.tensor_scalar(i0f[:], flo[:], -16.0, None, Alu.add)
            # valid masks from clips (integer-valued i0f):
            v0 = om.tile([128, 128], f32, tag="v0", name="v0")
            nc.vector.tensor_scalar(v0[:], i0f[:], 1.0, 1.0, Alu.add, Alu.min)
            nc.vector.tensor_scalar(v0[:], v0[:], 0.0, None, Alu.max)
            vtmp = om.tile([128, 128], f32, tag="vtmp", name="vtmp")
            nc.vector.tensor_scalar(vtmp[:], i0f[:], -1.0, 4096.0, Alu.mult,
                                    Alu.add)
            nc.vector.tensor_scalar(vtmp[:], vtmp[:], 1.0, 0.0, Alu.min, Alu.max)
            nc.vector.tensor_tensor(out=v0[:], in0=v0[:], in1=vtmp[:],
                                    op=Alu.mult)
            u0 = om.tile([128, 128], f32, tag="u0", name="u0")
            nc.vector.tensor_scalar(u0[:], i0f[:], 2.0, 1.0, Alu.add, Alu.min)
            nc.vector.tensor_scalar(u0[:], u0[:], 0.0, None, Alu.max)
            nc.vector.tensor_scalar(vtmp[:], i0f[:], -1.0, 4095.0, Alu.mult,
                                    Alu.add)
            nc.vector.tensor_scalar(vtmp[:], vtmp[:], 1.0, 0.0, Alu.min, Alu.max)
            nc.vector.tensor_tensor(out=u0[:], in0=u0[:], in1=vtmp[:],
                                    op=Alu.mult)
            w1p = om.tile([128, 128], f32, tag="w1p", name="w1p")
            nc.vector.tensor_tensor(out=w1p[:], in0=w1[:], in1=u0[:],
                                    op=Alu.mult)
            w0p = om.tile([128, 128], f32, tag="w0p", name="w0p")
            nc.vector.tensor_scalar(w0p[:], w1[:], -1.0, 1.0, Alu.mult, Alu.add)
            nc.vector.tensor_tensor(out=w0p[:], in0=w0p[:], in1=v0[:],
                                    op=Alu.mult)
            i0c = om.tile([128, 128], f32, tag="i0c", name="i0c")
            nc.vector.tensor_scalar(i0c[:], i0f[:], 0.0, 4095.0, Alu.max,
                                    Alu.min)
            i1c = om.tile([128, 128], f32, tag="i1c", name="i1c")
            nc.vector.tensor_scalar(i1c[:], i0f[:], 1.0, 0.0, Alu.add, Alu.max)
            nc.vector.tensor_scalar(i1c[:], i1c[:], 4095.0, None, Alu.min)
            # unfold into 8-row stacks for the PE wrap/weight transposes
            for srcf, dst, r0 in ((i0c, arri, 0), (i1c, arri, 4),
                                  (w0p, arrw, 0), (w1p, arrw, 4)):
                nc.sync.dma_start(
                    out=dst[r0 : r0 + 4, :].rearrange(
                        "g (lb li) -> g lb li", li=128),
                    in_=srcf[:],
                )

            # 16-wrapped int16 index layout for dma_gather: idx set r=(tap*4+g),
            # wrapped[p, k] = value at token 16k+p
            for pt4 in range(4):
                pw = ps_t.tile([16, 512], f32, tag="t", name="pw")
                for kk in range(64):
                    k = pt4 * 64 + kk
                    nc.tensor.transpose(
                        pw[:, kk * 8 : (kk + 1) * 8],
                        arri[:, k * 16 : (k + 1) * 16],
                        ident[0:8, 0:8],
                    )
                # chunks of C2 tokens = 16 k-blocks; 4 chunks per psum tile
                nc.vector.tensor_copy(
                    out=idxw[0:16, pt4 * 4 : (pt4 + 1) * 4, :, :],
                    in_=pw[:].rearrange("p (c kk st) -> p c st kk", c=4, kk=16,
                                        st=8),
                )
            for r in range(1, 8):
                nc.sync.dma_start(out=idxw[16 * r : 16 * (r + 1)], in_=idxw[0:16])

            # token-major bilinear weights: wtok[p, B, r] = arrw[r, 128B + p]
            pww = ps_t.tile([128, 512], f32, tag="t", name="pww")
            for b in range(32):
                nc.tensor.transpose(
                    pww[:, b * 8 : (b + 1) * 8],
                    arrw[:, b * 128 : (b + 1) * 128],
                    ident[0:8, 0:8],
                )
            nc.vector.tensor_copy(out=wtok[:], in_=pww[:, 0:256])

        # ------------------------- PASS 2 -------------------------
        nidx_reg = nc.gpsimd.to_reg(C2)
        with tc.tile_pool(name="p2", bufs=2) as p2, \
             tc.tile_pool(name="p2b", bufs=1) as p2b, \
             tc.tile_pool(name="p2g", bufs=4) as p2g:
            for c in range(NC2):
                l0 = c * C2
                qt2 = p2.tile([128, C2 // 128, D], bf16, tag="qt2", name="qt2")
                nc.sync.dma_start(
                    out=qt2[:],
                    in_=qT_d[l0 : l0 + C2, :].rearrange("(ls p) o -> p ls o", p=128),
                )
                relb = p2.tile([128, 8, C2], bf16, tag="relb", name="relb")
                nc.gpsimd.dma_start(out=relb[:], in_=relb_e[:, :, l0 : l0 + C2])
                xsT = p2.tile([128, C2 // 128, D], f32, tag="xsT", name="xsT")
                for g in range(G):
                    ga = p2g.tile([128, C2 // 128, 256], f32, tag="gth", name="ga")
                    nc.gpsimd.dma_gather(
                        out_ap=ga[:],
                        in_ap=x_e[:, g * 256 : (g + 1) * 256],
                        idxs_ap=idxw[:, c, g, :],
                        num_idxs=C2,
                        num_idxs_reg=nidx_reg,
                        elem_size=256,
                        elem_step=D,
                    )
                    gb = p2g.tile([128, C2 // 128, 256], f32, tag="gth", name="gb")
                    nc.gpsimd.dma_gather(
                        out_ap=gb[:],
                        in_ap=x_e[:, g * 256 : (g + 1) * 256],
                        idxs_ap=idxw[:, c, 4 + g, :],
                        num_idxs=C2,
                        num_idxs_reg=nidx_reg,
                        elem_size=256,
                        elem_step=D,
                    )
                    nb = C2 // 128
                    wa = wtok[:, nb * c : nb * (c + 1), g, None].to_broadcast(
                        (128, nb, 256))
                    wb = wtok[:, nb * c : nb * (c + 1), 4 + g, None].to_broadcast(
                        (128, nb, 256))
                    nc.vector.tensor_tensor(out=ga[:], in0=ga[:], in1=wa, op=Alu.mult)
                    nc.vector.tensor_tensor(out=gb[:], in0=gb[:], in1=wb, op=Alu.mult)
                    nc.vector.tensor_tensor(
                        out=xsT[:, :, g * 256 : (g + 1) * 256], in0=ga[:], in1=gb[:],
                        op=Alu.add,
                    )
                xs = p2b.tile([128, 8, C2], f32r, tag="xs", name="xs")
                for dc in range(8):
                    pt = ps_t.tile([128, 512], f32, tag="t", name="pt2")
                    for ls in range(C2 // 128):
                        nc.tensor.transpose(
                            pt[:, ls * 128 : (ls + 1) * 128],
                            xsT[:, ls, dc * 128 : (dc + 1) * 128],
                            ident[:],
                        )
                    nc.scalar.activation(out=xs[:, dc, :], in_=pt[:, 0:C2],
                                         func=Act.Copy)
                kt = p2b.tile([128, C2 // 128, D], bf16, tag="kt", name="kt")
                for ls in range(C2 // 128):
                    for oc in range(2):
                        pg = ps_g.tile([128, 512], f32, tag="g", name="pg2")
                        for dc in range(8):
                            nc.tensor.matmul(
                                pg[:],
                                lhsT=xs[:, dc, ls * 128 : (ls + 1) * 128],
                                rhs=wk[:, dc, oc * 512 : (oc + 1) * 512],
                                start=(dc == 0),
                                stop=(dc == 7),
                            )
                        nc.scalar.activation(
                            out=kt[:, ls, oc * 512 : (oc + 1) * 512], in_=pg[:],
                            func=Act.Copy,
                        )
                if has_bk:
                    for ls in range(C2 // 128):
                        nc.vector.tensor_tensor(
                            out=kt[:, ls, :], in0=kt[:, ls, :], in1=bkr[:], op=Alu.add
                        )
                vsb = p2.tile([128, 8, C2], f32r, tag="vsb", name="vsb")
                for dc in range(8):
                    pv = ps_v.tile([128, 512], f32, tag="v", name="pv")
                    for ds in range(8):
                        nc.tensor.matmul(
                            pv[:, :C2],
                            lhsT=wv[:, ds, dc * 128 : (dc + 1) * 128],
                            rhs=xs[:, ds, :],
                            start=(ds == 0),
                            stop=(ds == 7),
                        )
                    nc.vector.tensor_tensor(
                        out=vsb[:, dc, :], in0=pv[:, :C2], in1=relb[:, dc, :],
                        op=Alu.add,
                    )
                nc.sync.dma_start(out=v_d[:, :, l0 : l0 + C2], in_=vsb[:])
                # scores: sT[j, i] accumulated across all chunks in one PSUM bank
                # head-pair blocks: pair p covers channels [128p, 128p+128);
                # the [128,128] block has the two per-head scores on its
                # diagonal 64-blocks (off-diagonals are junk, zeroed later)
                for ls in range(C2 // 128):
                    for p in range(8):
                        sbank = score_e if p < 4 else score_o
                        pc = (p % 4) * 128
                        nc.tensor.matmul(
                            sbank[:, pc : pc + 128],
                            lhsT=kt[:, ls, 128 * p : 128 * p + 128],
                            rhs=qt2[:, ls, 128 * p : 128 * p + 128],
                            start=(c == 0 and ls == 0 and p % 4 == 0),
                            stop=(c == NC2 - 1 and ls == C2 // 128 - 1
                                  and p % 4 == 3),
                        )

        wo = wpool.tile([128, 8, D], f32r, tag="w", name="wo")
        nc.scalar.dma_start(out=wo, in_=wo_e[:])

        # ------------------------- SOFTMAX -------------------------
        esb = singles.tile([128, 1024], f32r, name="esb")
        nc.vector.memset(esb[:].bitcast(f32), 0.0)
        for h in range(H):
            p = h // 2
            sbank = score_e if p < 4 else score_o
            pc = (p % 4) * 128
            r0 = 64 * (h % 2)
            nc.scalar.activation(
                out=esb[r0 : r0 + 64, 128 * p + r0 : 128 * p + r0 + 64],
                in_=sbank[r0 : r0 + 64, pc + r0 : pc + r0 + 64],
                func=Act.Exp, scale=ATT_SCALE,
            )
        rs_ps = ps_g.tile([128, 16], f32, tag="g", name="rs_ps")
        for p in range(8):
            nc.tensor.matmul(
                rs_ps[:, 2 * p : 2 * p + 2],
                lhsT=esb[:, 128 * p : 128 * p + 128],
                rhs=ones2[:],
                start=True,
                stop=True,
            )
        rsi = singles.tile([128, 16], f32, name="rsi")
        nc.vector.reciprocal(out=rsi[:], in_=rs_ps[:])

        # ------------------------- PASS 3 -------------------------
        with tc.tile_pool(name="p3", bufs=2) as p3, tc.tile_pool(name="p3b", bufs=1) as p3b:
            for c in range(NC3):
                l0 = c * C3
                vs3 = p3.tile([128, 8, C3], f32r, tag="v3", name="vs3")
                nc.sync.dma_start(out=vs3[:], in_=v_d[:, :, l0 : l0 + C3])
                osb = p3b.tile([128, 8, C3], f32r, tag="o3", name="osb")
                for p in range(8):
                    po3 = ps_v.tile([128, 512], f32, tag="v", name="po3")
                    nc.tensor.matmul(
                        po3[:],
                        lhsT=esb[:, 128 * p : 128 * p + 128],
                        rhs=vs3[:, p, :],
                        start=True,
                        stop=True,
                    )
                    nc.scalar.activation(
                        out=osb[:, p, :], in_=po3[:], func=Act.Identity,
                        bias=0.0, scale=rsi[:, 2 * p : 2 * p + 1],
                    )
                yt = p3.tile([128, C3 // 128, D], f32, tag="yt", name="yt")
                for ls in range(C3 // 128):
                    for oc in range(2):
                        pg = ps_g.tile([128, 512], f32, tag="g", name="pg3")
                        for dc in range(8):
                            nc.tensor.matmul(
                                pg[:],
                                lhsT=osb[:, dc, ls * 128 : (ls + 1) * 128],
                                rhs=wo[:, dc, oc * 512 : (oc + 1) * 512],
                                start=(dc == 0),
                                stop=(dc == 7),
                            )
                        nc.vector.tensor_copy(
                            out=yt[:, ls, oc * 512 : (oc + 1) * 512], in_=pg[:]
                        )
                if has_bo:
                    for ls in range(C3 // 128):
                        nc.vector.tensor_tensor(
                            out=yt[:, ls, :], in0=yt[:, ls, :], in1=bor[:], op=Alu.add
                        )
                nc.sync.dma_start(
                    out=y_e[l0 : l0 + C3, :].rearrange("(ls p) o -> p ls o", p=128),
                    in_=yt[:],
                )

    nc.compile()
    return nc


_prog_cache: dict = {}


def get_program(has_bq: bool, has_bk: bool, has_bo: bool) -> bass.Bass:
    key = (has_bq, has_bk, has_bo)
    if key not in _prog_cache:
        _prog_cache[key] = build_program(*key)
    return _prog_cache[key]


def make_in_maps(inputs: dict) -> tuple[list[dict], tuple]:
    import ml_dtypes

    x = np.ascontiguousarray(np.asarray(inputs["x"], np.float32))
    Wq = np.asarray(inputs["Wq"], np.float32)
    bq = np.asarray(inputs["bq"], np.float32)
    Wk = np.asarray(inputs["Wk"], np.float32)
    bk = np.asarray(inputs["bk"], np.float32)
    Wv = np.asarray(inputs["Wv"], np.float32)
    bv = np.asarray(inputs["bv"], np.float32)
    Wo = np.asarray(inputs["Wo"], np.float32)
    bo = np.asarray(inputs["bo"], np.float32)
    Woff1 = np.asarray(inputs["Woff1"], np.float32)
    boff1 = np.asarray(inputs["boff1"], np.float32)
    Woff2 = np.asarray(inputs["Woff2"], np.float32)
    boff2 = np.asarray(inputs["boff2"], np.float32)
    rel_bias = np.asarray(inputs["rel_bias"], np.float32)

    def wts(w):  # (D, D) weight -> lhsT layout [128, 8, D]: [p, dc, o] = w[o, dc*128+p]
        return np.ascontiguousarray(
            w.T.reshape(8, 128, D).transpose(1, 0, 2).astype(np.float32)
        )

    # offset-path folding
    Weff = np.einsum("o,oit->it", Woff2[0].astype(np.float64),
                     Woff1.astype(np.float64))            # (256, 5)
    Poff = np.zeros((D, 128), np.float64)                 # padded to M=128 for fp32r
    offbias = np.zeros((20, 1), np.float64)
    for t in range(KW):
        for g in range(G):
            r = 4 * t + g
            Poff[:, r] = Weff[:, t] @ Wq[g * DG : (g + 1) * DG, :].astype(np.float64)
            offbias[r, 0] = Weff[:, t] @ bq[g * DG : (g + 1) * DG].astype(np.float64)
    c0 = float(Woff2[0].astype(np.float64) @ boff1.astype(np.float64)
               + np.float64(boff2[0]))
    Poffs = np.ascontiguousarray(
        Poff.reshape(8, 128, 128).transpose(1, 0, 2).astype(np.float32)
    )

    rel_eff = rel_bias[0] + bv[:, None]                   # (D, L)
    relb = np.ascontiguousarray(
        rel_eff.reshape(8, 128, L).transpose(1, 0, 2).astype(ml_dtypes.bfloat16)
    )

    ios16 = np.tile(
        (np.arange(L, dtype=np.float64) * S_NORM - 0.5 + 16.0)[None, :], (4, 1)
    ).astype(np.float32)
    c0vec = np.full((128, 1), c0, np.float32)
    pad2 = np.full((4, 2), float(boff2[0]), np.float32)

    has_bq = bool(np.any(bq != 0.0))
    has_bk = bool(np.any(bk != 0.0))
    has_bo = bool(np.any(bo != 0.0))

    shared = {
        "WqTs": wts(Wq),
        "WkTs": wts(Wk),
        "WvTs": wts(Wv),
        "WoTs": wts(Wo),
        "Poffs": Poffs,
        "offbias": offbias.astype(np.float32),
        "relb": relb,
        "ios16": ios16,
        "c0vec": c0vec,
        "pad2": pad2,
        "ones2": np.ones((128, 2), np.float32),
    }
    if has_bq:
        shared["bqrep"] = np.ascontiguousarray(np.tile(bq[None, :], (128, 1)))
    if has_bk:
        shared["bkrep"] = np.ascontiguousarray(np.tile(bk[None, :], (128, 1)))
    if has_bo:
        shared["borep"] = np.ascontiguousarray(np.tile(bo[None, :], (128, 1)))

    in_maps = [dict(shared, x=np.ascontiguousarray(x[i])) for i in range(NCORES)]
    return in_maps, (has_bq, has_bk, has_bo)


def kernel_run(inputs: dict, trace: bool = False):
    in_maps, flags = make_in_maps(inputs)
    nc = get_program(*flags)
    res = run_bass_kernel_spmd(nc, in_maps, list(range(NCORES)), trace=trace)
    y = np.stack([res.results[i]["y"] for i in range(NCORES)], axis=0)
    return y, res


def kernel(**inputs) -> np.ndarray:
    y, _ = kernel_run(inputs, trace=False)
    return y


# revision 18
# speedup vs baseline: 1.2282x; 1.0014x over previous
# Trainium2 Bass kernel for nn_CrossDeformAttn (deformable cross attention).
#
# Math (per batch b):
#   xc = x^T                                  (D, L) channel-major
#   q  = Wq xc + bq                           (D, L)
#   offset path: conv(q grouped, Woff1) -> Woff2 -> tanh*K -> bilinear sample
#     positions, sample xc per group -> xs    (D, L)
#   k = Wk xs + bk ; v = Wv xs + bv + rel_bias
#   per head (64 ch):  sT = k_h^T q_h summed over L   (64x64)
#   attn = softmax_j(sT * D^-0.5) ; out_h = attn @ v_h ; y = out^T Wo^T + bo
#
# Key host-side folding: the whole offset conv collapses.  With
#   Weff[i,t] = sum_o Woff2[o] Woff1[o,i,t]        (256, 5)
#   wproj[g,t,:] = Weff[:,t]^T @ Wq[g*256:(g+1)*256, :]   -> Poff (D, 20)
# the pre-tanh offset signal is a 20-channel projection of x plus shifted
# sums -- no 85 GFLOP conv on device.
#
# Device pipeline per core (1 batch element per core, 8 cores):
#   pass 1: stream x chunks -> PE-transpose -> xc -> qT (token-major, spill to
#           DRAM) + off_pre (20, L) -> shifted-tap accumulate (DMA accum)
#   offset math: tanh -> positions -> floor/frac (via fmod) -> clamped int16
#           indices (16-wrapped via tiny PE transposes) + bilinear weights
#           (token-major via PE transposes)
#   pass 2: dma_gather rows of x from HBM (2 taps x 4 groups), bilinear
#           combine token-major, PE-transpose -> xs, K GEMM -> kT, V GEMM ->
#           v (+rel_bias, spill), score matmuls accumulate in PSUM
#   softmax: exp on ACT (no max-sub needed; logits ~ +-5), row sums via
#           matmul with ones, reciprocal
#   pass 3: out_h = E^T.T @ v_h scaled by 1/rowsum, final Wo GEMM -> y
#
# All GEMMs run with float32r (FP22 reads, full PE rate at N>=256).

import sys

if "/opt/trn_rl_repo" not in sys.path:
    sys.path.insert(0, "/opt/trn_rl_repo")

from contextlib import ExitStack

import numpy as np

import concourse.bass as bass
import concourse.tile as tile
from concourse import bacc
from concourse import mybir
from concourse.bass_utils import run_bass_kernel_spmd
from concourse.masks import make_identity

f32 = mybir.dt.float32
f32r = mybir.dt.float32r
bf16 = mybir.dt.bfloat16
i16 = mybir.dt.int16
Alu = mybir.AluOpType
Act = mybir.ActivationFunctionType

B, L, D = 8, 4096, 1024
H, G, KW = 16, 4, 5
DG, DH = D // G, D // H
NCORES = 8

C1 = 512            # pass-1 chunk (tokens)
NC1 = L // C1
C2 = 256            # pass-2 chunk
NC2 = L // C2
C3 = 512            # pass-3 chunk
NC3 = L // C3

S_NORM = float(L) / float(L + 3)   # position scale from grid_sample algebra
ATT_SCALE = 1.0 / float(D) ** 0.5


def build_program(has_bq: bool, has_bk: bool, has_bo: bool) -> bass.Bass:
    nc = bacc.Bacc("TRN2", debug=False)

    x_e = nc.declare_dram_parameter("x", [L, D], f32, isOutput=False)
    wq_e = nc.declare_dram_parameter("WqTs", [128, 8, D], f32r, isOutput=False)
    wk_e = nc.declare_dram_parameter("WkTs", [128, 8, D], f32r, isOutput=False)
    wv_e = nc.declare_dram_parameter("WvTs", [128, 8, D], f32r, isOutput=False)
    wo_e = nc.declare_dram_parameter("WoTs", [128, 8, D], f32r, isOutput=False)
    poff_e = nc.declare_dram_parameter("Poffs", [128, 8, 128], f32r, isOutput=False)
    ones_e = nc.declare_dram_parameter("ones2", [128, 2], f32r, isOutput=False)
    offb_e = nc.declare_dram_parameter("offbias", [20, 1], f32, isOutput=False)
    relb_e = nc.declare_dram_parameter("relb", [128, 8, L], bf16, isOutput=False)
    ios_e = nc.declare_dram_parameter("ios16", [4, L], f32, isOutput=False)
    c0_e = nc.declare_dram_parameter("c0vec", [128, 1], f32, isOutput=False)
    pad2_e = nc.declare_dram_parameter("pad2", [4, 2], f32, isOutput=False)
    if has_bq:
        bq_e = nc.declare_dram_parameter("bqrep", [128, D], f32, isOutput=False)
    if has_bk:
        bk_e = nc.declare_dram_parameter("bkrep", [128, D], f32, isOutput=False)
    if has_bo:
        bo_e = nc.declare_dram_parameter("borep", [128, D], f32, isOutput=False)
    y_e = nc.declare_dram_parameter("y", [L, D], f32, isOutput=True)

    qT_d = nc.dram_tensor("qT_scratch", [L, D], bf16)
    v_d = nc.dram_tensor("v_scratch", [128, 8, L], f32r)

    with tile.TileContext(nc) as tc, ExitStack() as ctx:
        singles = ctx.enter_context(tc.tile_pool(name="singles", bufs=1))
        wpool = ctx.enter_context(tc.tile_pool(name="wpool", bufs=2))
        ps_t = ctx.enter_context(tc.tile_pool(name="ps_t", bufs=2, space="PSUM"))
        ps_g = ctx.enter_context(tc.tile_pool(name="ps_g", bufs=2, space="PSUM"))
        ps_v = ctx.enter_context(tc.tile_pool(name="ps_v", bufs=2, space="PSUM"))
        ps_s = ctx.enter_context(tc.tile_pool(name="ps_s", bufs=1, space="PSUM"))

        ident = singles.tile([128, 128], f32, name="ident")
        make_identity(nc, ident)

        wq = wpool.tile([128, 8, D], f32r, tag="w", name="wq")
        nc.scalar.dma_start(out=wq, in_=wq_e[:])
        poffs = singles.tile([128, 8, 128], f32r, name="poffs")
        ones2 = singles.tile([128, 2], f32r, name="ones2")
        nc.sync.dma_start(out=ones2, in_=ones_e[:])
        nc.sync.dma_start(out=poffs, in_=poff_e[:])
        offb = singles.tile([20, 1], f32, name="offb")
        nc.sync.dma_start(out=offb, in_=offb_e[:])
        c0v = singles.tile([128, 1], f32, name="c0v")
        nc.sync.dma_start(out=c0v, in_=c0_e[:])
        if has_bq:
            bqr = singles.tile([128, D], f32, name="bqr")
            nc.sync.dma_start(out=bqr, in_=bq_e[:])
        if has_bk:
            bkr = singles.tile([128, D], f32, name="bkr")
            nc.sync.dma_start(out=bkr, in_=bk_e[:])
        if has_bo:
            bor = singles.tile([128, D], f32, name="bor")
            nc.sync.dma_start(out=bor, in_=bo_e[:])

        # pre-tanh offset accumulator over the 5 conv taps (pad 8 absorbs the
        # shifted accumulate windows; cols >= L are spillover junk)
        offraw = singles.tile([4, L + 8], f32, name="offraw")
        nc.vector.memset(offraw[:], 0.0)

        wtok = singles.tile([128, 32, 8], f32, name="wtok")
        idxw = singles.tile([128, NC2, 8, C2 // 16], i16, name="idxw")
        # separate PSUM banks for even/odd heads so each bank has exactly one
        # accumulation-group start/stop
        score_e = ps_s.tile([128, 512], f32, tag="score_e", name="score_e")
        score_o = ps_s.tile([128, 512], f32, tag="score_o", name="score_o")

        # ------------------------- PASS 1 -------------------------
        with tc.tile_pool(name="p1", bufs=2) as p1, \
             tc.tile_pool(name="p1o", bufs=2) as p1o:
            for c in range(NC1):
                l0 = c * C1
                xt = p1.tile([128, C1 // 128, D], f32, tag="xt", name="xt")
                for ls in range(C1 // 128):
                    nc.sync.dma_start(
                        out=xt[:, ls, :],
                        in_=x_e[l0 + 128 * ls : l0 + 128 * (ls + 1), :].rearrange(
                            "p d -> p d"),
                    )
                xc = p1.tile([128, 8, C1], f32r, tag="xc", name="xc")
                for dc in range(8):
                    pt = ps_t.tile([128, 512], f32, tag="t", name="pt")
                    for ls in range(C1 // 128):
                        nc.tensor.transpose(
                            pt[:, ls * 128 : (ls + 1) * 128],
                            xt[:, ls, dc * 128 : (dc + 1) * 128],
                            ident[:],
                        )
                    nc.scalar.activation(out=xc[:, dc, :], in_=pt[:], func=Act.Copy)
                qt = p1.tile([128, C1 // 128, D], bf16, tag="qt", name="qt")
                for ls in range(C1 // 128):
                    for oc in range(2):
                        pg = ps_g.tile([128, 512], f32, tag="g", name="pg")
                        for dc in range(8):
                            nc.tensor.matmul(
                                pg[:],
                                lhsT=xc[:, dc, ls * 128 : (ls + 1) * 128],
                                rhs=wq[:, dc, oc * 512 : (oc + 1) * 512],
                                start=(dc == 0),
                                stop=(dc == 7),
                            )
                        nc.vector.tensor_copy(
                            out=qt[:, ls, oc * 512 : (oc + 1) * 512], in_=pg[:]
                        )
                if has_bq:
                    for ls in range(C1 // 128):
                        nc.vector.tensor_tensor(
                            out=qt[:, ls, :], in0=qt[:, ls, :], in1=bqr[:], op=Alu.add
                        )
                nc.sync.dma_start(
                    out=qT_d[l0 : l0 + C1, :].rearrange("(ls p) o -> p ls o", p=128),
                    in_=qt[:],
                )
                # off_pre: (20, C1) = Poff^T @ xc_chunk
                po = ps_v.tile([128, 512], f32, tag="v", name="po")
                for dc in range(8):
                    nc.tensor.matmul(
                        po[:, :C1],
                        lhsT=poffs[:, dc, :],
                        rhs=xc[:, dc, :],
                        start=(dc == 0),
                        stop=(dc == 7),
                    )
                offsb = p1o.tile([20, C1], f32, tag="offsb", name="offsb")
                nc.scalar.activation(
                    out=offsb[:], in_=po[:20, :C1], func=Act.Identity,
                    bias=offb[:], scale=1.0,
                )
                # shifted tap accumulate: off_pre[4t+g, l'] adds into
                # offraw[g, l' + 4 - t]
                for t in range(KW):
                    nc.gpsimd.dma_start(
                        out=offraw[0:4, l0 + 4 - t : l0 + 4 - t + C1],
                        in_=offsb[4 * t : 4 * t + 4, :],
                        accum_op=Alu.add,
                    )

        wk = wpool.tile([128, 8, D], f32r, tag="w", name="wk")
        nc.scalar.dma_start(out=wk, in_=wk_e[:])
        wv = wpool.tile([128, 8, D], f32r, tag="w", name="wv")
        nc.scalar.dma_start(out=wv, in_=wv_e[:])

        # ------------------------- OFFSET MATH -------------------------
        # Fold [4, L] -> [128, 128] (partition = g*32 + l//128) so the
        # elementwise chain uses all DVE lanes; fold/unfold are cheap DMAs.
        # arri rows: 0-3 = i0 clamped per group, 4-7 = i1 clamped
        # arrw rows: 0-3 = w0' (=(1-frac)*valid0), 4-7 = w1' (=frac*valid1)
        with tc.tile_pool(name="om", bufs=1) as om, \
             tc.tile_pool(name="omb", bufs=1) as omb:
            arri = omb.tile([8, L], f32, name="arri")
            arrw = omb.tile([8, L], f32, name="arrw")
            zf = om.tile([128, 128], f32, tag="zf", name="zf")
            nc.scalar.dma_start(
                out=zf[:],
                in_=offraw[0:4, 0:L].rearrange("g (lb li) -> g lb li", li=128),
            )
            iosb = om.tile([128, 128], f32, tag="iosb", name="iosb")
            nc.scalar.dma_start(out=iosb[:], in_=ios_e[:].rearrange(
                "g (lb li) -> g lb li", li=128))
            nc.vector.tensor_scalar(zf[:], zf[:], c0v[:, 0:1], None, Alu.add)
            # reference: offset for l<2 is exactly boff2 (the concat pad);
            # l in {0,1} sits at partitions g*32, cols 0:2
            for g in range(G):
                nc.scalar.dma_start(out=zf[32 * g : 32 * g + 1, 0:2],
                                    in_=pad2_e[g : g + 1, :])
            th = om.tile([128, 128], f32, tag="th", name="th")
            nc.scalar.activation(out=th[:], in_=zf[:], func=Act.Tanh)
            posp = om.tile([128, 128], f32, tag="posp", name="posp")
            # posp = 16 + pos = KW*tanh(z)*S + (iota*S - 0.5 + 16)
            nc.vector.tensor_scalar(posp[:], th[:], float(KW) * S_NORM, None,
                                    Alu.mult)
            nc.vector.tensor_tensor(out=posp[:], in0=posp[:], in1=iosb[:],
                                    op=Alu.add)
            i16t = om.tile([128, 128], i16, tag="i16t", name="i16t")
            nc.vector.tensor_copy(out=i16t[:], in_=posp[:])
            flo = om.tile([128, 128], f32, tag="flo", name="flo")
            nc.vector.tensor_copy(out=flo[:], in_=i16t[:])
            # rounding-mode-agnostic floor: flo -= (flo > posp)
            corr = om.tile([128, 128], f32, tag="corr", name="corr")
            nc.vector.tensor_tensor(out=corr[:], in0=flo[:], in1=posp[:],
                                    op=Alu.subtract)
            nc.vector.tensor_scalar(corr[:], corr[:], float(2 ** 25), 1.0,
                                    Alu.mult, Alu.min)
            nc.vector.tensor_scalar(corr[:], corr[:], 0.0, None, Alu.max)
            nc.vector.tensor_tensor(out=flo[:], in0=flo[:], in1=corr[:],
                                    op=Alu.subtract)
            w1 = om.tile([128, 128], f32, tag="w1", name="w1")
            nc.vector.tensor_tensor(out=w1[:], in0=posp[:], in1=flo[:],
                                    op=Alu.subtract)
            i0f = om.tile([128, 128], f32, tag="i0f", name="i0f")
            nc.vector.tensor_scalar(i0f[:], flo[:], -16.0, None, Alu.add)
            # valid masks from clips (integer-valued i0f):
            v0 = om.tile([128, 128], f32, tag="v0", name="v0")
            nc.vector.tensor_scalar(v0[:], i0f[:], 1.0, 1.0, Alu.add, Alu.min)
            nc.vector.tensor_scalar(v0[:], v0[:], 0.0, None, Alu.max)
            vtmp = om.tile([128, 128], f32, tag="vtmp", name="vtmp")
            nc.vector.tensor_scalar(vtmp[:], i0f[:], -1.0, 4096.0, Alu.mult,
                                    Alu.add)
            nc.vector.tensor_scalar(vtmp[:], vtmp[:], 1.0, 0.0, Alu.min, Alu.max)
            nc.vector.tensor_tensor(out=v0[:], in0=v0[:], in1=vtmp[:],
                                    op=Alu.mult)
            u0 = om.tile([128, 128], f32, tag="u0", name="u0")
            nc.vector.tensor_scalar(u0[:], i0f[:], 2.0, 1.0, Alu.add, Alu.min)
            nc.vector.tensor_scalar(u0[:], u0[:], 0.0, None, Alu.max)
            nc.vector.tensor_scalar(vtmp[:], i0f[:], -1.0, 4095.0, Alu.mult,
                                    Alu.add)
            nc.vector.tensor_scalar(vtmp[:], vtmp[:], 1.0, 0.0, Alu.min, Alu.max)
            nc.vector.tensor_tensor(out=u0[:], in0=u0[:], in1=vtmp[:],
                                    op=Alu.mult)
            w1p = om.tile([128, 128], f32, tag="w1p", name="w1p")
            nc.vector.tensor_tensor(out=w1p[:], in0=w1[:], in1=u0[:],
                                    op=Alu.mult)
            w0p = om.tile([128, 128], f32, tag="w0p", name="w0p")
            nc.vector.tensor_scalar(w0p[:], w1[:], -1.0, 1.0, Alu.mult, Alu.add)
            nc.vector.tensor_tensor(out=w0p[:], in0=w0p[:], in1=v0[:],
                                    op=Alu.mult)
            i0c = om.tile([128, 128], f32, tag="i0c", name="i0c")
            nc.vector.tensor_scalar(i0c[:], i0f[:], 0.0, 4095.0, Alu.max,
                                    Alu.min)
            i1c = om.tile([128, 128], f32, tag="i1c", name="i1c")
            nc.vector.tensor_scalar(i1c[:], i0f[:], 1.0, 0.0, Alu.add, Alu.max)
            nc.vector.tensor_scalar(i1c[:], i1c[:], 4095.0, None, Alu.min)
            # unfold into 8-row stacks for the PE wrap/weight transposes
            for srcf, dst, r0 in ((i0c, arri, 0), (i1c, arri, 4),
                                  (w0p, arrw, 0), (w1p, arrw, 4)):
                nc.scalar.dma_start(
                    out=dst[r0 : r0 + 4, :].rearrange(
                        "g (lb li) -> g lb li", li=128),
                    in_=srcf[:],
                )

            # 16-wrapped int16 index layout for dma_gather: idx set r=(tap*4+g),
            # wrapped[p, k] = value at token 16k+p
            for pt4 in range(4):
                pw = ps_t.tile([16, 512], f32, tag="t", name="pw")
                for kk in range(64):
                    k = pt4 * 64 + kk
                    nc.tensor.transpose(
                        pw[:, kk * 8 : (kk + 1) * 8],
                        arri[:, k * 16 : (k + 1) * 16],
                        ident[0:8, 0:8],
                    )
                # chunks of C2 tokens = 16 k-blocks; 4 chunks per psum tile
                nc.vector.tensor_copy(
                    out=idxw[0:16, pt4 * 4 : (pt4 + 1) * 4, :, :],
                    in_=pw[:].rearrange("p (c kk st) -> p c st kk", c=4, kk=16,
                                        st=8),
                )
            nc.scalar.dma_start(out=idxw[16:32], in_=idxw[0:16])
            nc.scalar.dma_start(out=idxw[32:64], in_=idxw[0:32])
            nc.scalar.dma_start(out=idxw[64:128], in_=idxw[0:64])

            # token-major bilinear weights: wtok[p, B, r] = arrw[r, 128B + p]
            pww = ps_t.tile([128, 512], f32, tag="t", name="pww")
            for b in range(32):
                nc.tensor.transpose(
                    pww[:, b * 8 : (b + 1) * 8],
                    arrw[:, b * 128 : (b + 1) * 128],
                    ident[0:8, 0:8],
                )
            nc.vector.tensor_copy(out=wtok[:], in_=pww[:, 0:256])

        # ------------------------- PASS 2 -------------------------
        nidx_reg = nc.gpsimd.to_reg(C2)
        with tc.tile_pool(name="p2", bufs=2) as p2, \
             tc.tile_pool(name="p2b", bufs=1) as p2b, \
             tc.tile_pool(name="p2g", bufs=4) as p2g:
            for c in range(NC2):
                l0 = c * C2
                qt2 = p2.tile([128, C2 // 128, D], bf16, tag="qt2", name="qt2")
                for ls in range(C2 // 128):
                    nc.sync.dma_start(
                        out=qt2[:, ls, :],
                        in_=qT_d[l0 + 128 * ls : l0 + 128 * (ls + 1), :],
                    )
                relb = p2.tile([128, 8, C2], bf16, tag="relb", name="relb")
                nc.gpsimd.dma_start(out=relb[:], in_=relb_e[:, :, l0 : l0 + C2])
                xsT = p2.tile([128, C2 // 128, D], f32, tag="xsT", name="xsT")
                for g in range(G):
                    ga = p2g.tile([128, C2 // 128, 256], f32, tag="gth", name="ga")
                    nc.gpsimd.dma_gather(
                        out_ap=ga[:],
                        in_ap=x_e[:, g * 256 : (g + 1) * 256],
                        idxs_ap=idxw[:, c, g, :],
                        num_idxs=C2,
                        num_idxs_reg=nidx_reg,
                        elem_size=256,
                        elem_step=D,
                    )
                    gb = p2g.tile([128, C2 // 128, 256], f32, tag="gth", name="gb")
                    nc.gpsimd.dma_gather(
                        out_ap=gb[:],
                        in_ap=x_e[:, g * 256 : (g + 1) * 256],
                        idxs_ap=idxw[:, c, 4 + g, :],
                        num_idxs=C2,
                        num_idxs_reg=nidx_reg,
                        elem_size=256,
                        elem_step=D,
                    )
                    nb = C2 // 128
                    wa = wtok[:, nb * c : nb * (c + 1), g, None].to_broadcast(
                        (128, nb, 256))
                    wb = wtok[:, nb * c : nb * (c + 1), 4 + g, None].to_broadcast(
                        (128, nb, 256))
                    nc.vector.tensor_tensor(out=ga[:], in0=ga[:], in1=wa, op=Alu.mult)
                    nc.vector.tensor_tensor(out=gb[:], in0=gb[:], in1=wb, op=Alu.mult)
                    nc.vector.tensor_tensor(
                        out=xsT[:, :, g * 256 : (g + 1) * 256], in0=ga[:], in1=gb[:],
                        op=Alu.add,
                    )
                xs = p2b.tile([128, 8, C2], f32r, tag="xs", name="xs")
                for dc in range(8):
                    pt = ps_t.tile([128, 512], f32, tag="t", name="pt2")
                    for ls in range(C2 // 128):
                        nc.tensor.transpose(
                            pt[:, ls * 128 : (ls + 1) * 128],
                            xsT[:, ls, dc * 128 : (dc + 1) * 128],
                            ident[:],
                        )
                    nc.scalar.activation(out=xs[:, dc, :], in_=pt[:, 0:C2],
                                         func=Act.Copy)
                kt = p2b.tile([128, C2 // 128, D], bf16, tag="kt", name="kt")
                for ls in range(C2 // 128):
                    for oc in range(2):
                        pg = ps_g.tile([128, 512], f32, tag="g", name="pg2")
                        for dc in range(8):
                            nc.tensor.matmul(
                                pg[:],
                                lhsT=xs[:, dc, ls * 128 : (ls + 1) * 128],
                                rhs=wk[:, dc, oc * 512 : (oc + 1) * 512],
                                start=(dc == 0),
                                stop=(dc == 7),
                            )
                        nc.scalar.activation(
                            out=kt[:, ls, oc * 512 : (oc + 1) * 512], in_=pg[:],
                            func=Act.Copy,
                        )
                if has_bk:
                    for ls in range(C2 // 128):
                        nc.vector.tensor_tensor(
                            out=kt[:, ls, :], in0=kt[:, ls, :], in1=bkr[:], op=Alu.add
                        )
                vsb = p2.tile([128, 8, C2], f32r, tag="vsb", name="vsb")
                for dc in range(8):
                    pv = ps_v.tile([128, 512], f32, tag="v", name="pv")
                    for ds in range(8):
                        nc.tensor.matmul(
                            pv[:, :C2],
                            lhsT=wv[:, ds, dc * 128 : (dc + 1) * 128],
                            rhs=xs[:, ds, :],
                            start=(ds == 0),
                            stop=(ds == 7),
                        )
                    nc.vector.tensor_tensor(
                        out=vsb[:, dc, :], in0=pv[:, :C2], in1=relb[:, dc, :],
                        op=Alu.add,
                    )
                nc.sync.dma_start(out=v_d[:, :, l0 : l0 + C2], in_=vsb[:])
                # scores: sT[j, i] accumulated across all chunks in one PSUM bank
                # head-pair blocks: pair p covers channels [128p, 128p+128);
                # the [128,128] block has the two per-head scores on its
                # diagonal 64-blocks (off-diagonals are junk, zeroed later)
                for ls in range(C2 // 128):
                    for p in range(8):
                        sbank = score_e if p < 4 else score_o
                        pc = (p % 4) * 128
                        nc.tensor.matmul(
                            sbank[:, pc : pc + 128],
                            lhsT=kt[:, ls, 128 * p : 128 * p + 128],
                            rhs=qt2[:, ls, 128 * p : 128 * p + 128],
                            start=(c == 0 and ls == 0 and p % 4 == 0),
                            stop=(c == NC2 - 1 and ls == C2 // 128 - 1
                                  and p % 4 == 3),
                        )

        wo = wpool.tile([128, 8, D], f32r, tag="w", name="wo")
        nc.scalar.dma_start(out=wo, in_=wo_e[:])

        # ------------------------- SOFTMAX -------------------------
        esb = singles.tile([128, 1024], f32r, name="esb")
        nc.vector.memset(esb[:].bitcast(f32), 0.0)
        for h in range(H):
            p = h // 2
            sbank = score_e if p < 4 else score_o
            pc = (p % 4) * 128
            r0 = 64 * (h % 2)
            nc.scalar.activation(
                out=esb[r0 : r0 + 64, 128 * p + r0 : 128 * p + r0 + 64],
                in_=sbank[r0 : r0 + 64, pc + r0 : pc + r0 + 64],
                func=Act.Exp, scale=ATT_SCALE,
            )
        rs_ps = ps_g.tile([128, 16], f32, tag="g", name="rs_ps")
        for p in range(8):
            nc.tensor.matmul(
                rs_ps[:, 2 * p : 2 * p + 2],
                lhsT=esb[:, 128 * p : 128 * p + 128],
                rhs=ones2[:],
                start=True,
                stop=True,
            )
        rsi = singles.tile([128, 16], f32, name="rsi")
        nc.vector.reciprocal(out=rsi[:], in_=rs_ps[:])

        # ------------------------- PASS 3 -------------------------
        with tc.tile_pool(name="p3", bufs=2) as p3, tc.tile_pool(name="p3b", bufs=1) as p3b:
            for c in range(NC3):
                l0 = c * C3
                vs3 = p3.tile([128, 8, C3], f32r, tag="v3", name="vs3")
                nc.sync.dma_start(out=vs3[:, 0:4, :], in_=v_d[:, 0:4, l0 : l0 + C3])
                nc.sync.dma_start(out=vs3[:, 4:8, :], in_=v_d[:, 4:8, l0 : l0 + C3])
                osb = p3b.tile([128, 8, C3], f32r, tag="o3", name="osb")
                for p in range(8):
                    po3 = ps_v.tile([128, 512], f32, tag="v", name="po3")
                    nc.tensor.matmul(
                        po3[:],
                        lhsT=esb[:, 128 * p : 128 * p + 128],
                        rhs=vs3[:, p, :],
                        start=True,
                        stop=True,
                    )
                    nc.scalar.activation(
                        out=osb[:, p, :], in_=po3[:], func=Act.Identity,
                        bias=0.0, scale=rsi[:, 2 * p : 2 * p + 1],
                    )
                yt = p3.tile([128, C3 // 128, D], f32, tag="yt", name="yt")
                for ls in range(C3 // 128):
                    for oc in range(2):
                        pg = ps_g.tile([128, 512], f32, tag="g", name="pg3")
                        for dc in range(8):
                            nc.tensor.matmul(
                                pg[:],
                                lhsT=osb[:, dc, ls * 128 : (ls + 1) * 128],
                                rhs=wo[:, dc, oc * 512 : (oc + 1) * 512],
                                start=(dc == 0),
                                stop=(dc == 7),
                            )
                        nc.vector.tensor_copy(
                            out=yt[:, ls, oc * 512 : (oc + 1) * 512], in_=pg[:]
                        )
                if has_bo:
                    for ls in range(C3 // 128):
                        nc.vector.tensor_tensor(
                            out=yt[:, ls, :], in0=yt[:, ls, :], in1=bor[:], op=Alu.add
                        )
                nc.sync.dma_start(
                    out=y_e[l0 : l0 + C3, :].rearrange("(ls p) o -> p ls o", p=128),
                    in_=yt[:],
                )

    nc.compile()
    return nc


_prog_cache: dict = {}


def get_program(has_bq: bool, has_bk: bool, has_bo: bool) -> bass.Bass:
    key = (has_bq, has_bk, has_bo)
    if key not in _prog_cache:
        _prog_cache[key] = build_program(*key)
    return _prog_cache[key]


def make_in_maps(inputs: dict) -> tuple[list[dict], tuple]:
    import ml_dtypes

    x = np.ascontiguousarray(np.asarray(inputs["x"], np.float32))
    Wq = np.asarray(inputs["Wq"], np.float32)
    bq = np.asarray(inputs["bq"], np.float32)
    Wk = np.asarray(inputs["Wk"], np.float32)
    bk = np.asarray(inputs["bk"], np.float32)
    Wv = np.asarray(inputs["Wv"], np.float32)
    bv = np.asarray(inputs["bv"], np.float32)
    Wo = np.asarray(inputs["Wo"], np.float32)
    bo = np.asarray(inputs["bo"], np.float32)
    Woff1 = np.asarray(inputs["Woff1"], np.float32)
    boff1 = np.asarray(inputs["boff1"], np.float32)
    Woff2 = np.asarray(inputs["Woff2"], np.float32)
    boff2 = np.asarray(inputs["boff2"], np.float32)
    rel_bias = np.asarray(inputs["rel_bias"], np.float32)

    def wts(w):  # (D, D) weight -> lhsT layout [128, 8, D]: [p, dc, o] = w[o, dc*128+p]
        return np.ascontiguousarray(
            w.T.reshape(8, 128, D).transpose(1, 0, 2).astype(np.float32)
        )

    # offset-path folding
    Weff = np.einsum("o,oit->it", Woff2[0].astype(np.float64),
                     Woff1.astype(np.float64))            # (256, 5)
    Poff = np.zeros((D, 128), np.float64)                 # padded to M=128 for fp32r
    offbias = np.zeros((20, 1), np.float64)
    for t in range(KW):
        for g in range(G):
            r = 4 * t + g
            Poff[:, r] = Weff[:, t] @ Wq[g * DG : (g + 1) * DG, :].astype(np.float64)
            offbias[r, 0] = Weff[:, t] @ bq[g * DG : (g + 1) * DG].astype(np.float64)
    c0 = float(Woff2[0].astype(np.float64) @ boff1.astype(np.float64)
               + np.float64(boff2[0]))
    Poffs = np.ascontiguousarray(
        Poff.reshape(8, 128, 128).transpose(1, 0, 2).astype(np.float32)
    )

    rel_eff = rel_bias[0] + bv[:, None]                   # (D, L)
    relb = np.ascontiguousarray(
        rel_eff.reshape(8, 128, L).transpose(1, 0, 2).astype(ml_dtypes.bfloat16)
    )

    ios16 = np.tile(
        (np.arange(L, dtype=np.float64) * S_NORM - 0.5 + 16.0)[None, :], (4, 1)
    ).astype(np.float32)
    c0vec = np.full((128, 1), c0, np.float32)
    pad2 = np.full((4, 2), float(boff2[0]), np.float32)

    has_bq = bool(np.any(bq != 0.0))
    has_bk = bool(np.any(bk != 0.0))
    has_bo = bool(np.any(bo != 0.0))

    shared = {
        "WqTs": wts(Wq),
        "WkTs": wts(Wk),
        "WvTs": wts(Wv),
        "WoTs": wts(Wo),
        "Poffs": Poffs,
        "offbias": offbias.astype(np.float32),
        "relb": relb,
        "ios16": ios16,
        "c0vec": c0vec,
        "pad2": pad2,
        "ones2": np.ones((128, 2), np.float32),
    }
    if has_bq:
        shared["bqrep"] = np.ascontiguousarray(np.tile(bq[None, :], (128, 1)))
    if has_bk:
        shared["bkrep"] = np.ascontiguousarray(np.tile(bk[None, :], (128, 1)))
    if has_bo:
        shared["borep"] = np.ascontiguousarray(np.tile(bo[None, :], (128, 1)))

    in_maps = [dict(shared, x=np.ascontiguousarray(x[i])) for i in range(NCORES)]
    return in_maps, (has_bq, has_bk, has_bo)


def kernel_run(inputs: dict, trace: bool = False):
    in_maps, flags = make_in_maps(inputs)
    nc = get_program(*flags)
    res = run_bass_kernel_spmd(nc, in_maps, list(range(NCORES)), trace=trace)
    y = np.stack([res.results[i]["y"] for i in range(NCORES)], axis=0)
    return y, res


def kernel(**inputs) -> np.ndarray:
    y, _ = kernel_run(inputs, trace=False)
    return y
